# revision 16
# baseline (speedup 1.0000x reference)
"""3-layer GAT on 8 trn2 NeuronCores (v2).

Strategy
--------
Nodes are permuted (snake-deal by in-degree, per-core degree sort) so each
core owns a contiguous range of 6272 padded ids (6250 real).  One Bass
program runs 3 times (one launch per GAT layer); the host concatenates
per-core outputs between launches.

Per-core ROTATION: core c's xt input is rotated so its own nodes sit at
table rows [0, 6272).  This makes every core's self-loop rows (= its dst
rows) a compile-time-static contiguous range fetched with a regular DMA,
and drops the self slots from the gather entirely.

Per launch, each core:
  1. BN-affine + relu on xt [128, 50176] -> h | al_s | al_d table
     T [50176, 256] bf16 (512B rows) in DRAM, batched copies (1024 rows
     per DMA).
  2. Blocks of 128 dsts are packed into GROUPS (slot-budgeted).  Per
     group: ONE dma_gather per int16 window (lo [0,32K), hi [N-32K,N)),
     one strided DMA for the self rows, fused mask-add + LeakyReLU + exp
     (bf16), per-block segment denominators via strided tensor_reduce,
     in-place alpha multiply on the gathered tile, per-block strided
     tensor_reduce aggregation, self-row contribution, transpose +
     head-mix matmul (bias folded pre-mix), BN partial sums per group.
Pad dst columns output exactly beta (bias); the host subtracts their
contribution from the BN partials and overwrites pad xt columns with
-bv/av so pad table rows have h = 0.
"""
import os
import numpy as np

import concourse.bass as bass
import concourse.bacc as bacc
import concourse.mybir as mybir
import concourse.tile as tile
from concourse import bass_utils
from concourse.masks import make_identity
from concourse.tile_sem_assignment import PROC_NAME_TO_IDX

_IDX_TO_PROC = {v: k for k, v in PROC_NAME_TO_IDX.items()}


def _bc(ap, pos, count):
    """Insert a step-0 (broadcast) axis into an AP at position pos."""
    lst = [list(x) for x in ap.ap]
    lst.insert(pos, [0, count])
    return bass.AP(ap.tensor, ap.offset, lst)


F32 = mybir.dt.float32
BF16 = mybir.dt.bfloat16
I16 = mybir.dt.int16

N = 50000
E = 800000
H = 2
CH = 64
IN = 128
OUT = 64
EPS = 1e-5
SLOPE = 0.2
NEG = -30000.0

N_CORES = 8
PER_CORE = 6272            # 49 * 128
NPAD = N_CORES * PER_CORE  # 50176
NBLK = PER_CORE // 128     # 49
REAL_PER_CORE = N // N_CORES  # 6250
NPAD_PER_CORE = PER_CORE - REAL_PER_CORE  # 22
D = 128                    # h channels
ROWE = 256                 # table row elems (bf16) = 512B; [h|als|ald|0..]
LO_END = 32768             # lo window [0, LO_END)
HI_START = NPAD - 32768    # hi window [HI_START, NPAD)
NQ = 4
S_CAP = 80                 # max slots (lo+hi) per group
NB_CAP = 8                 # max blocks per group


# ----------------------------------------------------------------- host prep

def _wrap_idxs(flat):
    """flat [n] int -> dma_gather idx layout [128, n/16] int16 (wrapped in 16
    partitions, replicated across the 8 q7 core groups)."""
    n = flat.shape[0]
    w = flat.reshape(n // 16, 16).T.astype(np.int16)
    return np.tile(w, (8, 1))


def preprocess(edge_index):
    """Build node permutation, per-core rotated ELL grids and masks."""
    src = edge_index[0].astype(np.int64)
    dst = edge_index[1].astype(np.int64)

    indeg = np.bincount(dst, minlength=N)  # real edges only (self via DMA)
    # deal nodes to cores, balancing edges: sort by in-degree, snake-deal
    order = np.argsort(-indeg, kind="stable")
    r = np.arange(N) % (2 * N_CORES)
    core_r = np.where(r < N_CORES, r, 2 * N_CORES - 1 - r)
    core_of = np.empty(N, np.int32)
    core_of[order] = core_r

    def ranks_for(key2):
        new_id = np.empty(N, np.int64)
        for c in range(N_CORES):
            nodes = np.where(core_of == c)[0]
            if key2 is None:
                k = np.lexsort((nodes, -indeg[nodes]))
            else:
                k = np.lexsort((nodes, key2[nodes], -indeg[nodes]))
            new_id[nodes[k]] = c * PER_CORE + np.arange(len(nodes))
        return new_id

    new_id = ranks_for(None)
    # per-dst window-balance key w.r.t. the dst's own core rotation
    s_new = new_id[src]
    d_core = core_of[dst]
    rot_s = (s_new - d_core * PER_CORE) % NPAD
    must_lo_cnt = np.bincount(dst, weights=(rot_s < HI_START).astype(np.float64),
                              minlength=N)
    must_hi_cnt = np.bincount(dst, weights=(rot_s >= LO_END).astype(np.float64),
                              minlength=N)
    new_id = ranks_for(must_lo_cnt - must_hi_cnt)

    ns = new_id[src]
    nd = new_id[dst]
    o = np.argsort(nd, kind="stable")
    ns, nd = ns[o], nd[o]
    starts = np.searchsorted(nd, np.arange(NPAD))
    ends = np.searchsorted(nd, np.arange(NPAD) + 1)

    # per (core, block) window budgets; shared (cross-core max) shapes
    KLO = np.zeros(NBLK, np.int64)
    KHI = np.zeros(NBLK, np.int64)
    rows_all = {}
    for c in range(N_CORES):
        base_c = c * PER_CORE
        for b in range(NBLK):
            ml = 0
            mh = 0
            dmax = 0
            rows = []
            for p in range(128):
                d_node = base_c + b * 128 + p
                sl = (ns[starts[d_node]:ends[d_node]] - base_c) % NPAD
                lo_m = sl[sl < HI_START]
                hi_m = sl[sl >= LO_END]
                mid = sl[(sl >= HI_START) & (sl < LO_END)]
                rows.append((lo_m, hi_m, mid))
                ml = max(ml, len(lo_m))
                mh = max(mh, len(hi_m))
                dmax = max(dmax, len(sl))
            klo = ml
            khi = max(mh, dmax - klo)
            KLO[b] = max(KLO[b], klo)
            KHI[b] = max(KHI[b], khi)
            rows_all[(c, b)] = rows

    tot_slots = int((KLO + KHI).sum() * 128)
    tot_edges = E // N_CORES
    print(f"[prep] slots/core {tot_slots} vs edges/core ~{tot_edges} "
          f"(pad {tot_slots / tot_edges - 1:.1%})  K={int((KLO + KHI).sum())}")

    # group blocks under a slot budget
    groups = []
    cur = []
    cur_s = 0
    for b in range(NBLK):
        sb = int(KLO[b] + KHI[b])
        if cur and (cur_s + sb > S_CAP or len(cur) >= NB_CAP):
            groups.append(cur)
            cur = []
            cur_s = 0
        cur.append(b)
        cur_s += sb
    if cur:
        groups.append(cur)
    print(f"[prep] {len(groups)} groups, sizes {[len(g) for g in groups]}")

    olo = np.concatenate([[0], np.cumsum(KLO)]).astype(int)
    ohi = np.concatenate([[0], np.cumsum(KHI)]).astype(int)
    SLO, SHI = int(KLO.sum()), int(KHI.sum())

    grids_lo = np.zeros((N_CORES, 128, SLO), np.int64)
    grids_hi = np.zeros((N_CORES, 128, SHI), np.int64)
    mlo = np.full((N_CORES, 128, SLO), NEG, np.float32)
    mhi = np.full((N_CORES, 128, SHI), NEG, np.float32)
    for c in range(N_CORES):
        for b in range(NBLK):
            klo, khi = int(KLO[b]), int(KHI[b])
            rows = rows_all[(c, b)]
            for p in range(128):
                lo_m, hi_m, mid = rows[p]
                lo = list(lo_m)
                hi = list(hi_m)
                room = klo - len(lo)
                lo += list(mid[:room])
                hi += list(mid[room:])
                assert len(lo) <= klo and len(hi) <= khi
                g = grids_lo[c, p]
                g[olo[b]:olo[b] + len(lo)] = lo
                g[olo[b] + len(lo):olo[b + 1]] = lo[0] if lo else 0
                g2 = grids_hi[c, p]
                g2[ohi[b]:ohi[b] + len(hi)] = hi
                g2[ohi[b] + len(hi):ohi[b + 1]] = hi[0] if hi else HI_START
                mlo[c, p, olo[b]:olo[b] + len(lo)] = 0.0
                mhi[c, p, ohi[b]:ohi[b] + len(hi)] = 0.0

    # wrap indices for dma_gather: per group, blocks concatenated, slot-major
    glo_w = np.zeros((N_CORES, 128, 8 * SLO), np.int16)
    ghi_w = np.zeros((N_CORES, 128, 8 * SHI), np.int16)
    for c in range(N_CORES):
        for grp in groups:
            fl = []
            fh = []
            for b in grp:
                fl.append(grids_lo[c, :, olo[b]:olo[b + 1]].T.reshape(-1))
                fh.append((grids_hi[c, :, ohi[b]:ohi[b + 1]].T.reshape(-1)
                           - HI_START))
            fl = np.concatenate(fl) if fl else np.zeros(0, np.int64)
            fh = np.concatenate(fh) if fh else np.zeros(0, np.int64)
            b0, b1 = grp[0], grp[-1]
            if len(fl):
                glo_w[c, :, 8 * olo[b0]:8 * olo[b1 + 1]] = _wrap_idxs(fl)
            if len(fh):
                ghi_w[c, :, 8 * ohi[b0]:8 * ohi[b1 + 1]] = _wrap_idxs(fh)

    return dict(new_id=new_id, KLO=KLO.tolist(), KHI=KHI.tolist(),
                groups=groups, glo=glo_w, ghi=ghi_w, mlo=mlo, mhi=mhi)


# ----------------------------------------------------------------- builder

def build(KLO, KHI, groups):
    nc = bacc.Bacc(None, target_bir_lowering=False, debug=False,
                   num_devices=N_CORES, num_swdge_queues=NQ)
    SLO, SHI = sum(KLO), sum(KHI)
    olo = np.concatenate([[0], np.cumsum(KLO)]).astype(int).tolist()
    ohi = np.concatenate([[0], np.cumsum(KHI)]).astype(int).tolist()

    xt = nc.dram_tensor("xt", [128, NPAD], F32, kind="ExternalInput")
    part = nc.dram_tensor("part", [128, 16], F32, kind="ExternalInput")
    gvec = nc.dram_tensor("gvec", [128, 1], F32, kind="ExternalInput")
    bevec = nc.dram_tensor("bevec", [128, 1], F32, kind="ExternalInput")
    srel = nc.dram_tensor("srel", [128, 1], F32, kind="ExternalInput")
    wtmat = nc.dram_tensor("wtmat", [128, 128], F32, kind="ExternalInput")
    emat = nc.dram_tensor("emat", [128, ROWE], F32, kind="ExternalInput")
    mmat = nc.dram_tensor("mmat", [128, 128], F32, kind="ExternalInput")
    biasv = nc.dram_tensor("biasv", [128, 1], F32, kind="ExternalInput")
    glod = nc.dram_tensor("glo", [128, 8 * SLO], I16, kind="ExternalInput")
    ghid = nc.dram_tensor("ghi", [128, 8 * SHI], I16, kind="ExternalInput")
    mlod = nc.dram_tensor("mlo", [128, SLO], F32, kind="ExternalInput")
    mhid = nc.dram_tensor("mhi", [128, SHI], F32, kind="ExternalInput")

    outb = nc.dram_tensor("outb", [128, PER_CORE], F32, kind="ExternalOutput")
    parts = nc.dram_tensor("parts", [128, 2], F32, kind="ExternalOutput")

    # split table: lo window rows [0, LO_END), hi window rows [HI_START,
    # NPAD).  Overlap rows are written to both, which lets lo-window
    # gathers start before the full build finishes.
    tbl_lo = nc.dram_tensor("tbl_lo", [LO_END, ROWE], BF16)
    tbl_hi = nc.dram_tensor("tbl_hi", [NPAD - HI_START, ROWE], BF16)

    with tile.TileContext(nc) as tc:
        with (
            tc.tile_pool(name="const", bufs=1) as cpool,
            tc.tile_pool(name="norm", bufs=2) as npool,
            tc.tile_pool(name="tb", bufs=2) as tbpool,
            tc.tile_pool(name="grid", bufs=3) as grpool,
            tc.tile_pool(name="g", bufs=2) as gpool,
            tc.tile_pool(name="work", bufs=2) as wpool,
            tc.tile_pool(name="small", bufs=3) as spool,
            tc.tile_pool(name="acc", bufs=1) as apool,
            tc.tile_pool(name="ps", bufs=2, space="PSUM") as pspool,
            tc.tile_pool(name="psw", bufs=1, space="PSUM") as pswpool,
            tc.tile_pool(name="ps2", bufs=2, space="PSUM") as ps2pool,
        ):
            ident = cpool.tile([128, 128], F32, tag="ident")
            make_identity(nc, ident[:])

            # --- BN params ------------------------------------------------
            pt = cpool.tile([128, 16], F32, tag="pt")
            nc.sync.dma_start(pt[:], part.ap())
            gv = cpool.tile([128, 1], F32, tag="gv")
            nc.sync.dma_start(gv[:], gvec.ap())
            bev = cpool.tile([128, 1], F32, tag="bev")
            nc.sync.dma_start(bev[:], bevec.ap())
            sv = cpool.tile([128, 1], F32, tag="sv")
            nc.sync.dma_start(sv[:], srel.ap())

            s1 = cpool.tile([128, 1], F32, tag="s1")
            s2 = cpool.tile([128, 1], F32, tag="s2")
            nc.vector.reduce_sum(s1[:], pt[:, 0:8], axis=mybir.AxisListType.X)
            nc.vector.reduce_sum(s2[:], pt[:, 8:16], axis=mybir.AxisListType.X)
            mu = cpool.tile([128, 1], F32, tag="mu")
            nc.vector.tensor_scalar_mul(mu[:], s1[:], 1.0 / N)
            msq = cpool.tile([128, 1], F32, tag="msq")
            nc.vector.tensor_scalar_mul(msq[:], s2[:], 1.0 / N)
            var = cpool.tile([128, 1], F32, tag="var")
            nc.vector.tensor_tensor(out=var[:], in0=mu[:], in1=mu[:],
                                    op=mybir.AluOpType.mult)
            nc.vector.tensor_tensor(out=var[:], in0=msq[:], in1=var[:],
                                    op=mybir.AluOpType.subtract)
            sd = cpool.tile([128, 1], F32, tag="sd")
            epsT = cpool.tile([128, 1], F32, tag="epsT")
            nc.vector.memset(epsT[:], EPS)
            nc.scalar.activation(sd[:], var[:], mybir.ActivationFunctionType.Sqrt,
                                 bias=epsT[:], scale=1.0)
            ra = cpool.tile([128, 1], F32, tag="ra")
            nc.vector.reciprocal(ra[:], sd[:])
            av = cpool.tile([128, 1], F32, tag="av")
            nc.vector.tensor_tensor(out=av[:], in0=ra[:], in1=gv[:],
                                    op=mybir.AluOpType.mult)
            bv = cpool.tile([128, 1], F32, tag="bv")
            nc.vector.tensor_tensor(out=bv[:], in0=mu[:], in1=av[:],
                                    op=mybir.AluOpType.mult)
            nc.vector.tensor_tensor(out=bv[:], in0=bev[:], in1=bv[:],
                                    op=mybir.AluOpType.subtract)

            wtt = cpool.tile([128, 128], F32, tag="wtt")
            nc.sync.dma_start(wtt[:], wtmat.ap())
            emt = cpool.tile([128, ROWE], F32, tag="emt")
            nc.sync.dma_start(emt[:], emat.ap())
            wep = pswpool.tile([128, ROWE], F32, tag="wep", space="PSUM")
            nc.tensor.matmul(wep[:], lhsT=wtt[:], rhs=emt[:], start=True, stop=True)
            web = cpool.tile([128, ROWE], BF16, tag="web")
            nc.scalar.copy(web[:], wep[:])
            mm = cpool.tile([128, 128], F32, tag="mm")
            nc.sync.dma_start(mm[:], mmat.ap())
            bi = cpool.tile([128, 1], F32, tag="bi")
            nc.sync.dma_start(bi[:], biasv.ap())
            slp = cpool.tile([128, 1], F32, tag="slp")
            nc.vector.memset(slp[:], SLOPE)

            # --- table build: T[r] = prelu(av*x+bv)^T @ [W|a] -------------
            # lo-window chunks first so lo gathers can start early
            CH_N = 1024
            chunk_order = (list(range(0, LO_END, CH_N))
                           + list(range(LO_END, NPAD, CH_N)))
            for r0 in chunk_order:
                xn = npool.tile([128, CH_N], F32, tag="xn")
                nc.sync.dma_start(xn[:], xt.ap()[:, r0:r0 + CH_N])
                u = npool.tile([128, CH_N], BF16, tag="u")
                nc.scalar.activation(u[:], xn[:],
                                     mybir.ActivationFunctionType.Prelu,
                                     bias=bv[:], scale=av[:], alpha=sv[:])
                hbt = tbpool.tile([128, 8 * ROWE], BF16, tag="hbt")
                hbt3 = hbt[:].rearrange("p (i e) -> p i e", e=ROWE)
                for q in range(4):
                    hp = pspool.tile([128, 2 * ROWE], F32, tag="hp",
                                     space="PSUM")
                    hp3 = hp[:].rearrange("p (i e) -> p i e", e=ROWE)
                    for t in range(2):
                        rr = (2 * q + t) * 128
                        nc.tensor.matmul(hp3[:, t, :], lhsT=u[:, rr:rr + 128],
                                         rhs=web[:], start=True, stop=True)
                    nc.scalar.copy(hbt3[:, 2 * q:2 * q + 2, 0:D],
                                   hp3[:, :, 0:D])
                    alv = bass.AP(hbt.tensor,
                                  hbt[:].offset + (2 * q) * ROWE + 132,
                                  [list(hbt[:].ap[0]), [ROWE, 2],
                                   [1, 8]]).bitcast(F32)
                    nc.vector.tensor_copy(alv, hp3[:, :, D:D + 4])
                if r0 < LO_END:
                    out_ap = bass.AP(tbl_lo, r0 * ROWE,
                                     [[ROWE, 128], [128 * ROWE, 8], [1, ROWE]])
                    nc.sync.dma_start(out_ap, hbt3)
                if r0 + CH_N > HI_START:
                    out_ap = bass.AP(tbl_hi, (r0 - HI_START) * ROWE,
                                     [[ROWE, 128], [128 * ROWE, 8], [1, ROWE]])
                    nc.sync.dma_start(out_ap, hbt3)

            # --- per-group aggregation ------------------------------------
            pacc = apool.tile([128, 2], F32, tag="pacc")
            nc.vector.memset(pacc[:], 0.0)
            qn = 0
            for grp in groups:
                nB = len(grp)
                b0 = grp[0]
                klo_g = [KLO[b] for b in grp]
                khi_g = [KHI[b] for b in grp]
                S_lo = sum(klo_g)
                S_hi = sum(khi_g)
                clo = np.concatenate([[0], np.cumsum(klo_g)]).astype(int).tolist()
                chi = np.concatenate([[0], np.cumsum(khi_g)]).astype(int).tolist()

                # self rows: dst rows of this group's blocks (contiguous)
                st = spool.tile([128, nB * ROWE], BF16, tag="st")
                st3 = st[:].rearrange("p (i e) -> p i e", e=ROWE)
                in_ap = bass.AP(tbl_lo, (b0 * 128) * ROWE,
                                [[ROWE, 128], [128 * ROWE, nB], [1, ROWE]])
                nc.sync.dma_start(st3, in_ap)
                # self als/ald f32 views [p, nB, 2]
                st_als = bass.AP(st.tensor, st[:].offset + 132,
                                 [list(st[:].ap[0]), [ROWE, nB],
                                  [1, 4]]).bitcast(F32)[:, :, 0:2]
                st_ald = bass.AP(st.tensor, st[:].offset + 136,
                                 [list(st[:].ap[0]), [ROWE, nB],
                                  [1, 4]]).bitcast(F32)[:, :, 0:2]

                uu = wpool.tile([128, nB * D], F32, tag="uu")
                uu3 = uu[:].rearrange("p (i c) -> p i c", c=D)
                uh = wpool.tile([128, nB * D], F32, tag="uh")
                uh3 = uh[:].rearrange("p (i c) -> p i c", c=D)
                den = spool.tile([128, nB * H], F32, tag="den")
                den3 = den[:].rearrange("p (i h) -> p i h", h=H)
                dhi = spool.tile([128, nB * H], F32, tag="dhi")
                dhi3 = dhi[:].rearrange("p (i h) -> p i h", h=H)

                for wname, S_w, kw, cw, owin, gridt, maskt, uuf, uuo, \
                        denf, deno in (
                    ("lo", S_lo, klo_g, clo, int(olo[b0]), glod, mlod,
                     uu, uu3, den, den3),
                    ("hi", S_hi, khi_g, chi, int(ohi[b0]), ghid, mhid,
                     uh, uh3, dhi, dhi3),
                ):
                    if S_w == 0:
                        nc.vector.memset(uuf[:], 0.0)
                        nc.vector.memset(denf[:], 0.0)
                        continue
                    glt = grpool.tile([128, 8 * S_w], I16, tag=f"glt{wname}")
                    nc.sync.dma_start(
                        glt[:], gridt.ap()[:, 8 * owin:8 * (owin + S_w)])
                    mkt = grpool.tile([128, S_w], F32, tag=f"mkt{wname}")
                    nc.sync.dma_start(
                        mkt[:], maskt.ap()[:, owin:owin + S_w])

                    g = gpool.tile([128, S_w * ROWE], BF16, tag=f"g{wname}")
                    g3 = g[:].rearrange("p (k e) -> p k e", e=ROWE)
                    win = (tbl_lo.ap() if wname == "lo" else tbl_hi.ap())
                    nc.gpsimd.dma_gather(
                        out_ap=g3, in_ap=win, idxs_ap=glt[:],
                        num_idxs=128 * S_w, num_idxs_reg=128 * S_w,
                        elem_size=ROWE, single_packet=False,
                        queue_num=qn % NQ)
                    qn += 1

                    # ee = als + mask; per (block, head): prelu(+ald bias),
                    # exp with den accumulation
                    ee = wpool.tile([128, S_w * H], F32, tag=f"ee{wname}")
                    ee3 = ee[:].rearrange("p (k h) -> p k h", h=H)
                    g_als = bass.AP(g.tensor, g[:].offset + 132,
                                    [list(g[:].ap[0]), [ROWE, S_w],
                                     [1, 4]]).bitcast(F32)[:, :, 0:2]
                    nc.vector.tensor_tensor(out=ee3, in0=g_als,
                                            in1=_bc(mkt[:], 2, H),
                                            op=mybir.AluOpType.add)
                    ex = wpool.tile([128, S_w * H], BF16, tag=f"ex{wname}")
                    ex3 = ex[:].rearrange("p (k h) -> p k h", h=H)
                    for gi in range(nB):
                        kb = kw[gi]
                        if kb == 0:
                            nc.vector.memset(denf[:, gi * H:gi * H + H], 0.0)
                            continue
                        for hh in range(H):
                            ald_col = bass.AP(
                                st.tensor,
                                st[:].offset + 136 + gi * ROWE + 2 * hh,
                                [list(st[:].ap[0]), [1, 2]]).bitcast(F32)
                            nc.scalar.activation(
                                ee3[:, cw[gi]:cw[gi + 1], hh],
                                ee3[:, cw[gi]:cw[gi + 1], hh],
                                mybir.ActivationFunctionType.Prelu,
                                bias=ald_col, scale=1.0, alpha=slp[:])
                            nc.scalar.activation(
                                ex3[:, cw[gi]:cw[gi + 1], hh],
                                ee3[:, cw[gi]:cw[gi + 1], hh],
                                mybir.ActivationFunctionType.Exp,
                                accum_out=denf[:, gi * H + hh:
                                               gi * H + hh + 1])

                    # in-place alpha multiply on gathered h
                    gh = bass.AP(g.tensor, g[:].offset,
                                 [list(g[:].ap[0]), [ROWE, S_w], [CH, H],
                                  [1, CH]])
                    exb = bass.AP(ex.tensor, ex[:].offset,
                                  [list(ex[:].ap[0]), [H, S_w], [1, H],
                                   [0, CH]])
                    nc.vector.tensor_tensor(out=gh, in0=gh, in1=exb,
                                            op=mybir.AluOpType.mult)

                    # aggregate per block: uu[gi] = sum_j alpha*h
                    for gi in range(nB):
                        kb = kw[gi]
                        if kb == 0:
                            nc.vector.memset(uuo[:, gi, :], 0.0)
                            continue
                        ghv = bass.AP(g.tensor, g[:].offset + cw[gi] * ROWE,
                                      [list(g[:].ap[0]), [1, D], [ROWE, kb]])
                        nc.vector.tensor_reduce(
                            uuo[:, gi, :], ghv, axis=mybir.AxisListType.X,
                            op=mybir.AluOpType.add)

                # self contribution: e = als+ald, prelu, exp; den & numerator
                es = spool.tile([128, nB * H], F32, tag="es")
                es3 = es[:].rearrange("p (i h) -> p i h", h=H)
                nc.vector.tensor_tensor(out=es3, in0=st_als, in1=st_ald,
                                        op=mybir.AluOpType.add)
                nc.scalar.activation(es[:], es[:],
                                     mybir.ActivationFunctionType.Prelu,
                                     alpha=slp[:])
                exs = spool.tile([128, nB * H], BF16, tag="exs")
                exs3 = exs[:].rearrange("p (i h) -> p i h", h=H)
                nc.scalar.activation(exs[:], es[:],
                                     mybir.ActivationFunctionType.Exp)

                # den total + reciprocal
                nc.vector.tensor_tensor(out=den[:], in0=den[:], in1=dhi[:],
                                        op=mybir.AluOpType.add)
                nc.vector.tensor_tensor(out=den3, in0=den3, in1=exs3,
                                        op=mybir.AluOpType.add)
                rden = spool.tile([128, nB * H], F32, tag="rden")
                nc.vector.reciprocal(rden[:], den[:])
                rden3 = rden[:].rearrange("p (i h) -> p i h", h=H)

                # self numerator: st.h *= exs ; uu += uh + st.h
                sth = bass.AP(st.tensor, st[:].offset,
                              [list(st[:].ap[0]), [ROWE, nB], [CH, H],
                               [1, CH]])
                exsb = bass.AP(exs.tensor, exs[:].offset,
                               [list(exs[:].ap[0]), [H, nB], [1, H],
                                [0, CH]])
                nc.vector.tensor_tensor(out=sth, in0=sth, in1=exsb,
                                        op=mybir.AluOpType.mult)
                nc.vector.tensor_tensor(out=uu[:], in0=uu[:], in1=uh[:],
                                        op=mybir.AluOpType.add)
                sthv = bass.AP(st.tensor, st[:].offset,
                               [list(st[:].ap[0]), [ROWE, nB], [1, D]])
                nc.vector.tensor_tensor(out=uu3, in0=uu3, in1=sthv,
                                        op=mybir.AluOpType.add)

                # scale by 1/den
                uu4 = bass.AP(uu.tensor, uu[:].offset,
                              [list(uu[:].ap[0]), [D, nB], [CH, H], [1, CH]])
                rdb = bass.AP(rden.tensor, rden[:].offset,
                              [list(rden[:].ap[0]), [H, nB], [1, H], [0, CH]])
                nc.vector.tensor_tensor(out=uu4, in0=uu4, in1=rdb,
                                        op=mybir.AluOpType.mult)

                # tail per block: transpose, +bias, head-mix, collect
                obt = wpool.tile([128, nB * D], F32, tag="obt")
                obt3 = obt[:].rearrange("p (i c) -> p i c", c=D)
                for gi in range(nB):
                    utp = ps2pool.tile([128, 128], F32, tag="utp",
                                       space="PSUM")
                    nc.tensor.transpose(utp[:], uu3[:, gi, :], ident[:])
                    uts = spool.tile([128, 128], F32, tag="uts")
                    nc.vector.tensor_scalar(out=uts[:], in0=utp[:],
                                            scalar1=bi[:], scalar2=None,
                                            op0=mybir.AluOpType.add)
                    otp = ps2pool.tile([128, 128], F32, tag="otp",
                                       space="PSUM")
                    nc.tensor.matmul(otp[:], lhsT=mm[:], rhs=uts[:],
                                     start=True, stop=True)
                    nc.scalar.copy(obt3[:, gi, :], otp[:])

                # partials
                sq = wpool.tile([128, nB * D], F32, tag="sq")
                nc.scalar.square(sq[:], obt[:])
                rs = spool.tile([128, 2], F32, tag="rs")
                nc.vector.reduce_sum(rs[:, 0:1], obt[:],
                                     axis=mybir.AxisListType.X)
                nc.vector.reduce_sum(rs[:, 1:2], sq[:],
                                     axis=mybir.AxisListType.X)
                nc.vector.tensor_tensor(out=pacc[:], in0=pacc[:], in1=rs[:],
                                        op=mybir.AluOpType.add)

                nc.sync.dma_start(
                    outb.ap()[:, b0 * 128:b0 * 128 + nB * D], obt[:])

            nc.sync.dma_start(parts.ap(), pacc[:])

    # align each gather's SWDGE queue with its Tile-assigned DMASW sem lane
    for bb in nc.main_func.blocks:
        for ins in bb.instructions:
            if isinstance(ins, mybir.InstDMAGatherAnt):
                nm = _IDX_TO_PROC.get(ins.bass_scheduled_proc, "")
                if nm.startswith("DMASW"):
                    ins.queue_num = int(nm[5:]) % NQ

    nc.compile()
    return nc


# ----------------------------------------------------------------- driver

_TRACE = bool(os.environ.get("KERNEL_TRACE"))
LAST_EXEC_NS = []


def kernel(x, edge_index, W0, a_src0, a_dst0, b0, g0, be0,
           W1, a_src1, a_dst1, b1, g1, be1,
           W2, a_src2, a_dst2, b2):
    global LAST_EXEC_NS
    LAST_EXEC_NS = []
    prep = preprocess(np.asarray(edge_index))
    new_id = prep["new_id"]

    nc = build(prep["KLO"], prep["KHI"], prep["groups"])

    xp = np.zeros((NPAD, IN), np.float32)
    xp[new_id] = np.asarray(x, np.float32)

    eye = np.eye(128, dtype=np.float32)
    mix2 = np.zeros((128, 128), np.float32)
    mix2[0:64, 0:64] = 0.5 * np.eye(64)
    mix2[64:128, 0:64] = 0.5 * np.eye(64)

    layers = [
        dict(W=W0, a_src=a_src0, a_dst=a_dst0,
             beta=np.asarray(b0, np.float32),
             g=np.full(128, np.sqrt(EPS), np.float32),
             be=np.zeros(128, np.float32), s=1.0, mix=eye),
        dict(W=W1, a_src=a_src1, a_dst=a_dst1,
             beta=np.asarray(b1, np.float32),
             g=np.asarray(g0, np.float32), be=np.asarray(be0, np.float32),
             s=0.0, mix=eye),
        dict(W=W2, a_src=a_src2, a_dst=a_dst2,
             beta=np.concatenate([np.asarray(b2, np.float32),
                                  np.asarray(b2, np.float32)]),
             g=np.asarray(g1, np.float32), be=np.asarray(be1, np.float32),
             s=0.0, mix=mix2),
    ]

    # pad column ids (global padded coords), per core
    pad_cols = np.concatenate(
        [np.arange(c * PER_CORE + REAL_PER_CORE, (c + 1) * PER_CORE)
         for c in range(N_CORES)])

    xt_cur = np.ascontiguousarray(xp.T).astype(np.float32)  # [128, NPAD]
    part_cur = np.zeros((128, 16), np.float32)

    outf = None
    for li, L in enumerate(layers):
        emat = np.zeros((128, ROWE), np.float32)
        emat[:, 0:128] = np.eye(128, dtype=np.float32)
        a_s = np.asarray(L["a_src"], np.float32)
        a_d = np.asarray(L["a_dst"], np.float32)
        for hh in range(H):
            emat[hh * CH:(hh + 1) * CH, D + hh] = a_s[hh]
            emat[hh * CH:(hh + 1) * CH, D + H + hh] = a_d[hh]

        # host-side BN params (match device math) for pad column values
        if li == 0:
            av = np.ones(128, np.float32)
            bv = np.zeros(128, np.float32)
        else:
            mu = part_cur[:, 0:8].sum(axis=1) / N
            msq = part_cur[:, 8:16].sum(axis=1) / N
            var = msq - mu * mu
            av = L["g"] / np.sqrt(var + EPS)
            bv = L["be"] - mu * av
        xt_cur[:, pad_cols] = (-bv / av)[:, None]

        in_maps = []
        for c in range(N_CORES):
            xt_rot = np.roll(xt_cur, -c * PER_CORE, axis=1)
            in_maps.append(dict(
                xt=np.ascontiguousarray(xt_rot),
                part=part_cur,
                gvec=np.asarray(L["g"], np.float32).reshape(128, 1),
                bevec=np.asarray(L["be"], np.float32).reshape(128, 1),
                srel=np.full((128, 1), L["s"], np.float32),
                wtmat=np.ascontiguousarray(np.asarray(L["W"], np.float32).T),
                emat=emat,
                mmat=np.asarray(L["mix"], np.float32),
                biasv=L["beta"].reshape(128, 1),
                glo=prep["glo"][c],
                ghi=prep["ghi"][c],
                mlo=prep["mlo"][c],
                mhi=prep["mhi"][c],
            ))

        res = bass_utils.run_bass_kernel_spmd(
            nc, in_maps, core_ids=list(range(N_CORES)), trace=_TRACE)
        if _TRACE and res.exec_time_ns:
            LAST_EXEC_NS.append(res.exec_time_ns)

        xt_cur = np.concatenate(
            [np.asarray(res.results[c]["outb"], np.float32)
             for c in range(N_CORES)], axis=1)
        # partials: subtract the pad columns' exact beta contribution
        beta = L["beta"]
        part_pairs = [np.asarray(res.results[c]["parts"], np.float32)
                      for c in range(N_CORES)]
        sums = np.stack([p[:, 0] - NPAD_PER_CORE * beta
                         for p in part_pairs], axis=1)
        sqs = np.stack([p[:, 1] - NPAD_PER_CORE * beta * beta
                        for p in part_pairs], axis=1)
        part_cur = np.concatenate([sums, sqs], axis=1).astype(np.float32)
        if li == 2:
            outf = xt_cur

    out = np.zeros((N, OUT), np.float32)
    out[np.arange(N)] = outf[:OUT, :].T[new_id]
    return out


# revision 29
# speedup vs baseline: 1.4020x; 1.4020x over previous
"""3-layer GAT on 8 trn2 NeuronCores (v2).

Strategy
--------
Nodes are permuted (snake-deal by in-degree, per-core degree sort) so each
core owns a contiguous range of 6272 padded ids (6250 real).  One Bass
program runs 3 times (one launch per GAT layer); the host concatenates
per-core outputs between launches.

Per-core ROTATION: core c's xt input is rotated so its own nodes sit at
table rows [0, 6272).  This makes every core's self-loop rows (= its dst
rows) a compile-time-static contiguous range fetched with a regular DMA,
and drops the self slots from the gather entirely.

Per launch, each core:
  1. BN-affine + relu on xt [128, 50176] -> h | al_s | al_d table
     T [50176, 256] bf16 (512B rows) in DRAM, batched copies (1024 rows
     per DMA).
  2. Blocks of 128 dsts are packed into GROUPS (slot-budgeted).  Per
     group: ONE dma_gather per int16 window (lo [0,32K), hi [N-32K,N)),
     one strided DMA for the self rows, fused mask-add + LeakyReLU + exp
     (bf16), per-block segment denominators via strided tensor_reduce,
     in-place alpha multiply on the gathered tile, per-block strided
     tensor_reduce aggregation, self-row contribution, transpose +
     head-mix matmul (bias folded pre-mix), BN partial sums per group.
Pad dst columns output exactly beta (bias); the host subtracts their
contribution from the BN partials and overwrites pad xt columns with
-bv/av so pad table rows have h = 0.
"""
import os
import numpy as np

import concourse.bass as bass
import concourse.bacc as bacc
import concourse.mybir as mybir
import concourse.tile as tile
from concourse import bass_utils
from concourse.masks import make_identity
from concourse.tile_sem_assignment import PROC_NAME_TO_IDX

_IDX_TO_PROC = {v: k for k, v in PROC_NAME_TO_IDX.items()}


def _bc(ap, pos, count):
    """Insert a step-0 (broadcast) axis into an AP at position pos."""
    lst = [list(x) for x in ap.ap]
    lst.insert(pos, [0, count])
    return bass.AP(ap.tensor, ap.offset, lst)


F32 = mybir.dt.float32
BF16 = mybir.dt.bfloat16
I16 = mybir.dt.int16

N = 50000
E = 800000
H = 2
CH = 64
IN = 128
OUT = 64
EPS = 1e-5
SLOPE = 0.2
NEG = -30000.0

N_CORES = 8
PER_CORE = 6272            # 49 * 128
NPAD = N_CORES * PER_CORE  # 50176
NBLK = PER_CORE // 128     # 49
REAL_PER_CORE = N // N_CORES  # 6250
NPAD_PER_CORE = PER_CORE - REAL_PER_CORE  # 22
D = 128                    # h channels
ROWE = 256                 # table row elems (bf16) = 512B; [h|als|ald|0..]
LO_END = 32768             # lo window [0, LO_END)
HI_START = NPAD - 32768    # hi window [HI_START, NPAD)
NQ = 4
S_CAP = 64                 # max slots (lo+hi) per group
NB_CAP = 8                 # max blocks per group


# ----------------------------------------------------------------- host prep

def _wrap_idxs(flat):
    """flat [n] int -> dma_gather idx layout [128, n/16] int16 (wrapped in 16
    partitions, replicated across the 8 q7 core groups)."""
    n = flat.shape[0]
    w = flat.reshape(n // 16, 16).T.astype(np.int16)
    return np.tile(w, (8, 1))


def preprocess(edge_index):
    """Build node permutation, per-core rotated ELL grids and masks."""
    src = edge_index[0].astype(np.int64)
    dst = edge_index[1].astype(np.int64)

    indeg = np.bincount(dst, minlength=N)  # real edges only (self via DMA)
    # deal nodes to cores, balancing edges: sort by in-degree, snake-deal
    order = np.argsort(-indeg, kind="stable")
    r = np.arange(N) % (2 * N_CORES)
    core_r = np.where(r < N_CORES, r, 2 * N_CORES - 1 - r)
    core_of = np.empty(N, np.int32)
    core_of[order] = core_r

    def ranks_for(key2):
        new_id = np.empty(N, np.int64)
        for c in range(N_CORES):
            nodes = np.where(core_of == c)[0]
            if key2 is None:
                k = np.lexsort((nodes, -indeg[nodes]))
            else:
                k = np.lexsort((nodes, key2[nodes], -indeg[nodes]))
            new_id[nodes[k]] = c * PER_CORE + np.arange(len(nodes))
        return new_id

    new_id = ranks_for(None)
    # per-dst window-balance key w.r.t. the dst's own core rotation
    s_new = new_id[src]
    d_core = core_of[dst]
    rot_s = (s_new - d_core * PER_CORE) % NPAD
    must_lo_cnt = np.bincount(dst, weights=(rot_s < HI_START).astype(np.float64),
                              minlength=N)
    must_hi_cnt = np.bincount(dst, weights=(rot_s >= LO_END).astype(np.float64),
                              minlength=N)
    new_id = ranks_for(must_lo_cnt - must_hi_cnt)

    ns = new_id[src]
    nd = new_id[dst]
    o = np.argsort(nd, kind="stable")
    ns, nd = ns[o], nd[o]
    starts = np.searchsorted(nd, np.arange(NPAD))
    ends = np.searchsorted(nd, np.arange(NPAD) + 1)

    # per (core, block) window budgets; shared (cross-core max) shapes
    KLO = np.zeros(NBLK, np.int64)
    KHI = np.zeros(NBLK, np.int64)
    rows_all = {}
    for c in range(N_CORES):
        base_c = c * PER_CORE
        for b in range(NBLK):
            ml = 0
            mh = 0
            dmax = 0
            rows = []
            for p in range(128):
                d_node = base_c + b * 128 + p
                sl = (ns[starts[d_node]:ends[d_node]] - base_c) % NPAD
                lo_m = sl[sl < HI_START]
                hi_m = sl[sl >= LO_END]
                mid = sl[(sl >= HI_START) & (sl < LO_END)]
                rows.append((lo_m, hi_m, mid))
                ml = max(ml, len(lo_m))
                mh = max(mh, len(hi_m))
                dmax = max(dmax, len(sl))
            klo = ml
            khi = max(mh, dmax - klo)
            KLO[b] = max(KLO[b], klo)
            KHI[b] = max(KHI[b], khi)
            rows_all[(c, b)] = rows

    tot_slots = int((KLO + KHI).sum() * 128)
    tot_edges = E // N_CORES
    print(f"[prep] slots/core {tot_slots} vs edges/core ~{tot_edges} "
          f"(pad {tot_slots / tot_edges - 1:.1%})  K={int((KLO + KHI).sum())}")

    # group blocks under a slot budget
    groups = []
    cur = []
    cur_s = 0
    for b in range(NBLK):
        sb = int(KLO[b] + KHI[b])
        if cur and (cur_s + sb > S_CAP or len(cur) >= NB_CAP):
            groups.append(cur)
            cur = []
            cur_s = 0
        cur.append(b)
        cur_s += sb
    if cur:
        groups.append(cur)
    print(f"[prep] {len(groups)} groups, sizes {[len(g) for g in groups]}")

    olo = np.concatenate([[0], np.cumsum(KLO)]).astype(int)
    ohi = np.concatenate([[0], np.cumsum(KHI)]).astype(int)
    SLO, SHI = int(KLO.sum()), int(KHI.sum())

    grids_lo = np.zeros((N_CORES, 128, SLO), np.int64)
    grids_hi = np.zeros((N_CORES, 128, SHI), np.int64)
    mlo = np.full((N_CORES, 128, SLO), NEG, np.float32)
    mhi = np.full((N_CORES, 128, SHI), NEG, np.float32)
    for c in range(N_CORES):
        for b in range(NBLK):
            klo, khi = int(KLO[b]), int(KHI[b])
            rows = rows_all[(c, b)]
            for p in range(128):
                lo_m, hi_m, mid = rows[p]
                lo = list(lo_m)
                hi = list(hi_m)
                room = klo - len(lo)
                lo += list(mid[:room])
                hi += list(mid[room:])
                assert len(lo) <= klo and len(hi) <= khi
                g = grids_lo[c, p]
                g[olo[b]:olo[b] + len(lo)] = lo
                g[olo[b] + len(lo):olo[b + 1]] = lo[0] if lo else 0
                g2 = grids_hi[c, p]
                g2[ohi[b]:ohi[b] + len(hi)] = hi
                g2[ohi[b] + len(hi):ohi[b + 1]] = hi[0] if hi else HI_START
                mlo[c, p, olo[b]:olo[b] + len(lo)] = 0.0
                mhi[c, p, ohi[b]:ohi[b] + len(hi)] = 0.0

    # wrap indices for dma_gather: per group, blocks concatenated, slot-major
    glo_w = np.zeros((N_CORES, 128, 8 * SLO), np.int16)
    ghi_w = np.zeros((N_CORES, 128, 8 * SHI), np.int16)
    for c in range(N_CORES):
        for grp in groups:
            fl = []
            fh = []
            for b in grp:
                fl.append(grids_lo[c, :, olo[b]:olo[b + 1]].T.reshape(-1))
                fh.append((grids_hi[c, :, ohi[b]:ohi[b + 1]].T.reshape(-1)
                           - HI_START))
            fl = np.concatenate(fl) if fl else np.zeros(0, np.int64)
            fh = np.concatenate(fh) if fh else np.zeros(0, np.int64)
            b0, b1 = grp[0], grp[-1]
            if len(fl):
                glo_w[c, :, 8 * olo[b0]:8 * olo[b1 + 1]] = _wrap_idxs(fl)
            if len(fh):
                ghi_w[c, :, 8 * ohi[b0]:8 * ohi[b1 + 1]] = _wrap_idxs(fh)

    return dict(new_id=new_id, KLO=KLO.tolist(), KHI=KHI.tolist(),
                groups=groups, glo=glo_w, ghi=ghi_w, mlo=mlo, mhi=mhi)


# ----------------------------------------------------------------- builder

def build(KLO, KHI, groups):
    nc = bacc.Bacc(None, target_bir_lowering=False, debug=False,
                   num_devices=N_CORES, num_swdge_queues=NQ)
    SLO, SHI = sum(KLO), sum(KHI)
    olo = np.concatenate([[0], np.cumsum(KLO)]).astype(int).tolist()
    ohi = np.concatenate([[0], np.cumsum(KHI)]).astype(int).tolist()

    xt = nc.dram_tensor("xt", [128, NPAD], BF16, kind="ExternalInput")
    part = nc.dram_tensor("part", [128, 16], F32, kind="ExternalInput")
    gvec = nc.dram_tensor("gvec", [128, 1], F32, kind="ExternalInput")
    bevec = nc.dram_tensor("bevec", [128, 1], F32, kind="ExternalInput")
    srel = nc.dram_tensor("srel", [128, 1], F32, kind="ExternalInput")
    wtmat = nc.dram_tensor("wtmat", [128, 128], F32, kind="ExternalInput")
    emat = nc.dram_tensor("emat", [128, ROWE], F32, kind="ExternalInput")
    mmat = nc.dram_tensor("mmat", [128, 128], F32, kind="ExternalInput")
    biasv = nc.dram_tensor("biasv", [128, 1], F32, kind="ExternalInput")
    glod = nc.dram_tensor("glo", [128, 8 * SLO], I16, kind="ExternalInput")
    ghid = nc.dram_tensor("ghi", [128, 8 * SHI], I16, kind="ExternalInput")
    mlod = nc.dram_tensor("mlo", [128, SLO], F32, kind="ExternalInput")
    mhid = nc.dram_tensor("mhi", [128, SHI], F32, kind="ExternalInput")

    outb = nc.dram_tensor("outb", [128, PER_CORE], F32, kind="ExternalOutput")
    parts = nc.dram_tensor("parts", [128, 2], F32, kind="ExternalOutput")

    tbl = nc.dram_tensor("tbl", [NPAD, ROWE], BF16)  # internal

    with tile.TileContext(nc) as tc:
        with (
            tc.tile_pool(name="const", bufs=1) as cpool,
            tc.tile_pool(name="norm", bufs=2) as npool,
            tc.tile_pool(name="tb", bufs=2) as tbpool,
            tc.tile_pool(name="grid", bufs=4) as grpool,
            tc.tile_pool(name="glo", bufs=3) as glopool,
            tc.tile_pool(name="ghi", bufs=3) as ghipool,
            tc.tile_pool(name="self", bufs=6) as stpool,
            tc.tile_pool(name="work", bufs=2) as wpool,
            tc.tile_pool(name="small", bufs=3) as spool,
            tc.tile_pool(name="acc", bufs=1) as apool,
            tc.tile_pool(name="ps", bufs=2, space="PSUM") as pspool,
            tc.tile_pool(name="psw", bufs=1, space="PSUM") as pswpool,
            tc.tile_pool(name="ps2", bufs=2, space="PSUM") as ps2pool,
        ):
            ident = cpool.tile([128, 128], F32, tag="ident")
            make_identity(nc, ident[:])

            # --- BN params ------------------------------------------------
            pt = cpool.tile([128, 16], F32, tag="pt")
            nc.sync.dma_start(pt[:], part.ap())
            gv = cpool.tile([128, 1], F32, tag="gv")
            nc.sync.dma_start(gv[:], gvec.ap())
            bev = cpool.tile([128, 1], F32, tag="bev")
            nc.sync.dma_start(bev[:], bevec.ap())
            sv = cpool.tile([128, 1], F32, tag="sv")
            nc.sync.dma_start(sv[:], srel.ap())

            s1 = cpool.tile([128, 1], F32, tag="s1")
            s2 = cpool.tile([128, 1], F32, tag="s2")
            nc.vector.reduce_sum(s1[:], pt[:, 0:8], axis=mybir.AxisListType.X)
            nc.vector.reduce_sum(s2[:], pt[:, 8:16], axis=mybir.AxisListType.X)
            mu = cpool.tile([128, 1], F32, tag="mu")
            nc.vector.tensor_scalar_mul(mu[:], s1[:], 1.0 / N)
            msq = cpool.tile([128, 1], F32, tag="msq")
            nc.vector.tensor_scalar_mul(msq[:], s2[:], 1.0 / N)
            var = cpool.tile([128, 1], F32, tag="var")
            nc.vector.tensor_tensor(out=var[:], in0=mu[:], in1=mu[:],
                                    op=mybir.AluOpType.mult)
            nc.vector.tensor_tensor(out=var[:], in0=msq[:], in1=var[:],
                                    op=mybir.AluOpType.subtract)
            sd = cpool.tile([128, 1], F32, tag="sd")
            epsT = cpool.tile([128, 1], F32, tag="epsT")
            nc.vector.memset(epsT[:], EPS)
            nc.scalar.activation(sd[:], var[:], mybir.ActivationFunctionType.Sqrt,
                                 bias=epsT[:], scale=1.0)
            ra = cpool.tile([128, 1], F32, tag="ra")
            nc.vector.reciprocal(ra[:], sd[:])
            av = cpool.tile([128, 1], F32, tag="av")
            nc.vector.tensor_tensor(out=av[:], in0=ra[:], in1=gv[:],
                                    op=mybir.AluOpType.mult)
            bv = cpool.tile([128, 1], F32, tag="bv")
            nc.vector.tensor_tensor(out=bv[:], in0=mu[:], in1=av[:],
                                    op=mybir.AluOpType.mult)
            nc.vector.tensor_tensor(out=bv[:], in0=bev[:], in1=bv[:],
                                    op=mybir.AluOpType.subtract)

            wtt = cpool.tile([128, 128], F32, tag="wtt")
            nc.sync.dma_start(wtt[:], wtmat.ap())
            emt = cpool.tile([128, ROWE], F32, tag="emt")
            nc.sync.dma_start(emt[:], emat.ap())
            wep = pswpool.tile([128, ROWE], F32, tag="wep", space="PSUM")
            nc.tensor.matmul(wep[:], lhsT=wtt[:], rhs=emt[:], start=True, stop=True)
            web = cpool.tile([128, ROWE], BF16, tag="web")
            nc.scalar.copy(web[:], wep[:])
            mm = cpool.tile([128, 128], F32, tag="mm")
            nc.sync.dma_start(mm[:], mmat.ap())
            bi = cpool.tile([128, 1], F32, tag="bi")
            nc.sync.dma_start(bi[:], biasv.ap())
            slp = cpool.tile([128, 1], F32, tag="slp")
            nc.vector.memset(slp[:], SLOPE)

            # --- table build: T[r] = prelu(av*x+bv)^T @ [W|a] -------------
            # row layout (bf16): [h(128) | als(2) | ald(2) | pad]
            CH_N = 1024
            for r0 in range(0, NPAD, CH_N):
                xn = npool.tile([128, CH_N], BF16, tag="xn")
                nc.sync.dma_start(xn[:], xt.ap()[:, r0:r0 + CH_N])
                u = npool.tile([128, CH_N], BF16, tag="u")
                nc.scalar.activation(u[:], xn[:],
                                     mybir.ActivationFunctionType.Prelu,
                                     bias=bv[:], scale=av[:], alpha=sv[:])
                hbt = tbpool.tile([128, 8 * ROWE], BF16, tag="hbt")
                hbt3 = hbt[:].rearrange("p (i e) -> p i e", e=ROWE)
                for q in range(4):
                    hp = pspool.tile([128, 2 * ROWE], F32, tag="hp",
                                     space="PSUM")
                    hp3 = hp[:].rearrange("p (i e) -> p i e", e=ROWE)
                    for t in range(2):
                        rr = (2 * q + t) * 128
                        nc.tensor.matmul(hp3[:, t, :], lhsT=u[:, rr:rr + 128],
                                         rhs=web[:], start=True, stop=True)
                    nc.scalar.copy(hbt3[:, 2 * q:2 * q + 2, 0:D + 2 * H],
                                   hp3[:, :, 0:D + 2 * H])
                out_ap = bass.AP(tbl, r0 * ROWE,
                                 [[ROWE, 128], [128 * ROWE, 8], [1, ROWE]])
                nc.sync.dma_start(out_ap, hbt3)

            # --- per-group aggregation ------------------------------------
            pacc = apool.tile([128, 2], F32, tag="pacc")
            nc.vector.memset(pacc[:], 0.0)
            qn = 0
            for grp in groups:
                nB = len(grp)
                b0 = grp[0]
                klo_g = [KLO[b] for b in grp]
                khi_g = [KHI[b] for b in grp]
                S_lo = sum(klo_g)
                S_hi = sum(khi_g)
                clo = np.concatenate([[0], np.cumsum(klo_g)]).astype(int).tolist()
                chi = np.concatenate([[0], np.cumsum(khi_g)]).astype(int).tolist()

                # self rows: dst rows of this group's blocks (contiguous)
                st = stpool.tile([128, nB * ROWE], BF16, tag="st")
                st3 = st[:].rearrange("p (i e) -> p i e", e=ROWE)
                in_ap = bass.AP(tbl, (b0 * 128) * ROWE,
                                [[ROWE, 128], [128 * ROWE, nB], [1, ROWE]])
                nc.sync.dma_start(st3, in_ap)
                # self als/ald bf16 views [p, nB, 2]
                st_als = bass.AP(st.tensor, st[:].offset + D,
                                 [list(st[:].ap[0]), [ROWE, nB], [1, 2]])
                st_ald = bass.AP(st.tensor, st[:].offset + D + H,
                                 [list(st[:].ap[0]), [ROWE, nB], [1, 2]])

                uu = wpool.tile([128, nB * D], F32, tag="uu")
                uu3 = uu[:].rearrange("p (i c) -> p i c", c=D)
                uh = wpool.tile([128, nB * D], F32, tag="uh")
                uh3 = uh[:].rearrange("p (i c) -> p i c", c=D)
                den = spool.tile([128, nB * H], F32, tag="den")
                den3 = den[:].rearrange("p (i h) -> p i h", h=H)
                dhi = spool.tile([128, nB * H], F32, tag="dhi")
                dhi3 = dhi[:].rearrange("p (i h) -> p i h", h=H)

                for wname, S_w, kw, cw, owin, gridt, maskt, uuf, uuo, \
                        denf, deno in (
                    ("lo", S_lo, klo_g, clo, int(olo[b0]), glod, mlod,
                     uu, uu3, den, den3),
                    ("hi", S_hi, khi_g, chi, int(ohi[b0]), ghid, mhid,
                     uh, uh3, dhi, dhi3),
                ):
                    if S_w == 0:
                        nc.vector.memset(uuf[:], 0.0)
                        nc.vector.memset(denf[:], 0.0)
                        continue
                    glt = grpool.tile([128, 8 * S_w], I16, tag=f"glt{wname}")
                    nc.sync.dma_start(
                        glt[:], gridt.ap()[:, 8 * owin:8 * (owin + S_w)])
                    mkt = grpool.tile([128, S_w], F32, tag=f"mkt{wname}")
                    nc.sync.dma_start(
                        mkt[:], maskt.ap()[:, owin:owin + S_w])

                    g = (glopool if wname == "lo" else ghipool).tile(
                        [128, S_w * ROWE], BF16, tag=f"g{wname}")
                    g3 = g[:].rearrange("p (k e) -> p k e", e=ROWE)
                    win = (tbl.ap()[0:LO_END, :] if wname == "lo"
                           else tbl.ap()[HI_START:NPAD, :])
                    nc.gpsimd.dma_gather(
                        out_ap=g3, in_ap=win, idxs_ap=glt[:],
                        num_idxs=128 * S_w, num_idxs_reg=128 * S_w,
                        elem_size=ROWE, single_packet=False,
                        queue_num=qn % NQ)
                    qn += 1

                    # ee = als + mask (+ald per block); prelu; exp -> bf16
                    ee = wpool.tile([128, S_w * H], F32, tag=f"ee{wname}")
                    ee3 = ee[:].rearrange("p (k h) -> p k h", h=H)
                    g_als = bass.AP(g.tensor, g[:].offset + D,
                                    [list(g[:].ap[0]), [ROWE, S_w], [1, 2]])
                    nc.vector.tensor_tensor(out=ee3, in0=g_als,
                                            in1=_bc(mkt[:], 2, H),
                                            op=mybir.AluOpType.add)
                    for gi in range(nB):
                        kb = kw[gi]
                        if kb == 0:
                            continue
                        ald_b = bass.AP(st.tensor,
                                        st[:].offset + D + H + gi * ROWE,
                                        [list(st[:].ap[0]), [0, kb], [1, 2]])
                        nc.vector.tensor_tensor(
                            out=ee3[:, cw[gi]:cw[gi + 1], :],
                            in0=ee3[:, cw[gi]:cw[gi + 1], :],
                            in1=ald_b, op=mybir.AluOpType.add)
                    nc.scalar.activation(ee[:], ee[:],
                                         mybir.ActivationFunctionType.Prelu,
                                         alpha=slp[:])
                    ex = wpool.tile([128, S_w * H], BF16, tag=f"ex{wname}")
                    ex3 = ex[:].rearrange("p (k h) -> p k h", h=H)
                    nc.scalar.activation(ex[:], ee[:],
                                         mybir.ActivationFunctionType.Exp)
                    for gi in range(nB):
                        kb = kw[gi]
                        if kb == 0:
                            nc.vector.memset(denf[:, gi * H:gi * H + H], 0.0)
                            continue
                        exv = bass.AP(ex.tensor, ex[:].offset + cw[gi] * H,
                                      [list(ex[:].ap[0]), [1, H], [H, kb]])
                        nc.vector.tensor_reduce(
                            denf[:, gi * H:gi * H + H], exv,
                            axis=mybir.AxisListType.X,
                            op=mybir.AluOpType.add)

                    # in-place alpha multiply on gathered h
                    gh = bass.AP(g.tensor, g[:].offset,
                                 [list(g[:].ap[0]), [ROWE, S_w], [CH, H],
                                  [1, CH]])
                    exb = bass.AP(ex.tensor, ex[:].offset,
                                  [list(ex[:].ap[0]), [H, S_w], [1, H],
                                   [0, CH]])
                    nc.vector.tensor_tensor(out=gh, in0=gh, in1=exb,
                                            op=mybir.AluOpType.mult)

                    # aggregate per block: uu[gi] = sum_j alpha*h
                    for gi in range(nB):
                        kb = kw[gi]
                        if kb == 0:
                            nc.vector.memset(uuo[:, gi, :], 0.0)
                            continue
                        ghv = bass.AP(g.tensor, g[:].offset + cw[gi] * ROWE,
                                      [list(g[:].ap[0]), [1, D], [ROWE, kb]])
                        nc.vector.tensor_reduce(
                            uuo[:, gi, :], ghv, axis=mybir.AxisListType.X,
                            op=mybir.AluOpType.add)

                # self contribution: e = als+ald, prelu, exp; den & numerator
                es = spool.tile([128, nB * H], F32, tag="es")
                es3 = es[:].rearrange("p (i h) -> p i h", h=H)
                nc.vector.tensor_tensor(out=es3, in0=st_als, in1=st_ald,
                                        op=mybir.AluOpType.add)
                nc.scalar.activation(es[:], es[:],
                                     mybir.ActivationFunctionType.Prelu,
                                     alpha=slp[:])
                exs = spool.tile([128, nB * H], BF16, tag="exs")
                exs3 = exs[:].rearrange("p (i h) -> p i h", h=H)
                nc.scalar.activation(exs[:], es[:],
                                     mybir.ActivationFunctionType.Exp)

                # den total + reciprocal
                nc.vector.tensor_tensor(out=den[:], in0=den[:], in1=dhi[:],
                                        op=mybir.AluOpType.add)
                nc.vector.tensor_tensor(out=den3, in0=den3, in1=exs3,
                                        op=mybir.AluOpType.add)
                rden = spool.tile([128, nB * H], F32, tag="rden")
                nc.vector.reciprocal(rden[:], den[:])
                rden3 = rden[:].rearrange("p (i h) -> p i h", h=H)

                # self numerator: st.h *= exs ; uu += uh + st.h
                sth = bass.AP(st.tensor, st[:].offset,
                              [list(st[:].ap[0]), [ROWE, nB], [CH, H],
                               [1, CH]])
                exsb = bass.AP(exs.tensor, exs[:].offset,
                               [list(exs[:].ap[0]), [H, nB], [1, H],
                                [0, CH]])
                nc.vector.tensor_tensor(out=sth, in0=sth, in1=exsb,
                                        op=mybir.AluOpType.mult)
                nc.vector.tensor_tensor(out=uu[:], in0=uu[:], in1=uh[:],
                                        op=mybir.AluOpType.add)
                sthv = bass.AP(st.tensor, st[:].offset,
                               [list(st[:].ap[0]), [ROWE, nB], [1, D]])
                nc.vector.tensor_tensor(out=uu3, in0=uu3, in1=sthv,
                                        op=mybir.AluOpType.add)

                # scale by 1/den
                uu4 = bass.AP(uu.tensor, uu[:].offset,
                              [list(uu[:].ap[0]), [D, nB], [CH, H], [1, CH]])
                rdb = bass.AP(rden.tensor, rden[:].offset,
                              [list(rden[:].ap[0]), [H, nB], [1, H], [0, CH]])
                nc.vector.tensor_tensor(out=uu4, in0=uu4, in1=rdb,
                                        op=mybir.AluOpType.mult)

                # tail per block: transpose, +bias, head-mix, collect
                obt = wpool.tile([128, nB * D], F32, tag="obt")
                obt3 = obt[:].rearrange("p (i c) -> p i c", c=D)
                for gi in range(nB):
                    utp = ps2pool.tile([128, 128], F32, tag="utp",
                                       space="PSUM")
                    nc.tensor.transpose(utp[:], uu3[:, gi, :], ident[:])
                    uts = spool.tile([128, 128], F32, tag="uts")
                    nc.vector.tensor_scalar(out=uts[:], in0=utp[:],
                                            scalar1=bi[:], scalar2=None,
                                            op0=mybir.AluOpType.add)
                    otp = ps2pool.tile([128, 128], F32, tag="otp",
                                       space="PSUM")
                    nc.tensor.matmul(otp[:], lhsT=mm[:], rhs=uts[:],
                                     start=True, stop=True)
                    nc.scalar.copy(obt3[:, gi, :], otp[:])

                # partials
                sq = wpool.tile([128, nB * D], F32, tag="sq")
                nc.scalar.square(sq[:], obt[:])
                rs = spool.tile([128, 2], F32, tag="rs")
                nc.vector.reduce_sum(rs[:, 0:1], obt[:],
                                     axis=mybir.AxisListType.X)
                nc.vector.reduce_sum(rs[:, 1:2], sq[:],
                                     axis=mybir.AxisListType.X)
                nc.vector.tensor_tensor(out=pacc[:], in0=pacc[:], in1=rs[:],
                                        op=mybir.AluOpType.add)

                nc.sync.dma_start(
                    outb.ap()[:, b0 * 128:b0 * 128 + nB * D], obt[:])

            nc.sync.dma_start(parts.ap(), pacc[:])

    # align each gather's SWDGE queue with its Tile-assigned DMASW sem lane
    for bb in nc.main_func.blocks:
        for ins in bb.instructions:
            if isinstance(ins, mybir.InstDMAGatherAnt):
                nm = _IDX_TO_PROC.get(ins.bass_scheduled_proc, "")
                if nm.startswith("DMASW"):
                    ins.queue_num = int(nm[5:]) % NQ

    nc.compile()
    return nc


# ----------------------------------------------------------------- driver

_TRACE = bool(os.environ.get("KERNEL_TRACE"))
LAST_EXEC_NS = []


def kernel(x, edge_index, W0, a_src0, a_dst0, b0, g0, be0,
           W1, a_src1, a_dst1, b1, g1, be1,
           W2, a_src2, a_dst2, b2):
    global LAST_EXEC_NS
    LAST_EXEC_NS = []
    prep = preprocess(np.asarray(edge_index))
    new_id = prep["new_id"]

    nc = build(prep["KLO"], prep["KHI"], prep["groups"])

    xp = np.zeros((NPAD, IN), np.float32)
    xp[new_id] = np.asarray(x, np.float32)

    eye = np.eye(128, dtype=np.float32)
    mix2 = np.zeros((128, 128), np.float32)
    mix2[0:64, 0:64] = 0.5 * np.eye(64)
    mix2[64:128, 0:64] = 0.5 * np.eye(64)

    layers = [
        dict(W=W0, a_src=a_src0, a_dst=a_dst0,
             beta=np.asarray(b0, np.float32),
             g=np.full(128, np.sqrt(EPS), np.float32),
             be=np.zeros(128, np.float32), s=1.0, mix=eye),
        dict(W=W1, a_src=a_src1, a_dst=a_dst1,
             beta=np.asarray(b1, np.float32),
             g=np.asarray(g0, np.float32), be=np.asarray(be0, np.float32),
             s=0.0, mix=eye),
        dict(W=W2, a_src=a_src2, a_dst=a_dst2,
             beta=np.concatenate([np.asarray(b2, np.float32),
                                  np.asarray(b2, np.float32)]),
             g=np.asarray(g1, np.float32), be=np.asarray(be1, np.float32),
             s=0.0, mix=mix2),
    ]

    # pad column ids (global padded coords), per core
    pad_cols = np.concatenate(
        [np.arange(c * PER_CORE + REAL_PER_CORE, (c + 1) * PER_CORE)
         for c in range(N_CORES)])

    xt_cur = np.ascontiguousarray(xp.T).astype(np.float32)  # [128, NPAD]
    part_cur = np.zeros((128, 16), np.float32)

    outf = None
    for li, L in enumerate(layers):
        emat = np.zeros((128, ROWE), np.float32)
        emat[:, 0:128] = np.eye(128, dtype=np.float32)
        a_s = np.asarray(L["a_src"], np.float32)
        a_d = np.asarray(L["a_dst"], np.float32)
        for hh in range(H):
            emat[hh * CH:(hh + 1) * CH, D + hh] = a_s[hh]
            emat[hh * CH:(hh + 1) * CH, D + H + hh] = a_d[hh]

        # host-side BN params (match device math) for pad column values
        if li == 0:
            av = np.ones(128, np.float32)
            bv = np.zeros(128, np.float32)
        else:
            mu = part_cur[:, 0:8].sum(axis=1) / N
            msq = part_cur[:, 8:16].sum(axis=1) / N
            var = msq - mu * mu
            av = L["g"] / np.sqrt(var + EPS)
            bv = L["be"] - mu * av
        xt_cur[:, pad_cols] = (-bv / av)[:, None]

        import ml_dtypes
        in_maps = []
        for c in range(N_CORES):
            xt_rot = np.roll(xt_cur, -c * PER_CORE, axis=1)
            in_maps.append(dict(
                xt=np.ascontiguousarray(xt_rot).astype(ml_dtypes.bfloat16),
                part=part_cur,
                gvec=np.asarray(L["g"], np.float32).reshape(128, 1),
                bevec=np.asarray(L["be"], np.float32).reshape(128, 1),
                srel=np.full((128, 1), L["s"], np.float32),
                wtmat=np.ascontiguousarray(np.asarray(L["W"], np.float32).T),
                emat=emat,
                mmat=np.asarray(L["mix"], np.float32),
                biasv=L["beta"].reshape(128, 1),
                glo=prep["glo"][c],
                ghi=prep["ghi"][c],
                mlo=prep["mlo"][c],
                mhi=prep["mhi"][c],
            ))

        res = bass_utils.run_bass_kernel_spmd(
            nc, in_maps, core_ids=list(range(N_CORES)), trace=_TRACE)
        if _TRACE and res.exec_time_ns:
            LAST_EXEC_NS.append(res.exec_time_ns)

        xt_cur = np.concatenate(
            [np.asarray(res.results[c]["outb"], np.float32)
             for c in range(N_CORES)], axis=1)
        # partials: subtract the pad columns' exact beta contribution
        beta = L["beta"]
        part_pairs = [np.asarray(res.results[c]["parts"], np.float32)
                      for c in range(N_CORES)]
        sums = np.stack([p[:, 0] - NPAD_PER_CORE * beta
                         for p in part_pairs], axis=1)
        sqs = np.stack([p[:, 1] - NPAD_PER_CORE * beta * beta
                        for p in part_pairs], axis=1)
        part_cur = np.concatenate([sums, sqs], axis=1).astype(np.float32)
        if li == 2:
            outf = xt_cur

    out = np.zeros((N, OUT), np.float32)
    out[np.arange(N)] = outf[:OUT, :].T[new_id]
    return out


# revision 36
# speedup vs baseline: 1.4029x; 1.0006x over previous
"""3-layer GAT on 8 trn2 NeuronCores (v2).

Strategy
--------
Nodes are permuted (snake-deal by in-degree, per-core degree sort) so each
core owns a contiguous range of 6272 padded ids (6250 real).  One Bass
program runs 3 times (one launch per GAT layer); the host concatenates
per-core outputs between launches.

Per-core ROTATION: core c's xt input is rotated so its own nodes sit at
table rows [0, 6272).  This makes every core's self-loop rows (= its dst
rows) a compile-time-static contiguous range fetched with a regular DMA,
and drops the self slots from the gather entirely.

Per launch, each core:
  1. BN-affine + relu on xt [128, 50176] -> h | al_s | al_d table
     T [50176, 256] bf16 (512B rows) in DRAM, batched copies (1024 rows
     per DMA).
  2. Blocks of 128 dsts are packed into GROUPS (slot-budgeted).  Per
     group: ONE dma_gather per int16 window (lo [0,32K), hi [N-32K,N)),
     one strided DMA for the self rows, fused mask-add + LeakyReLU + exp
     (bf16), per-block segment denominators via strided tensor_reduce,
     in-place alpha multiply on the gathered tile, per-block strided
     tensor_reduce aggregation, self-row contribution, transpose +
     head-mix matmul (bias folded pre-mix), BN partial sums per group.
Pad dst columns output exactly beta (bias); the host subtracts their
contribution from the BN partials and overwrites pad xt columns with
-bv/av so pad table rows have h = 0.
"""
import os
import numpy as np

import concourse.bass as bass
import concourse.bacc as bacc
import concourse.mybir as mybir
import concourse.tile as tile
from concourse import bass_utils
from concourse.masks import make_identity
from concourse.tile_sem_assignment import PROC_NAME_TO_IDX

_IDX_TO_PROC = {v: k for k, v in PROC_NAME_TO_IDX.items()}


def _bc(ap, pos, count):
    """Insert a step-0 (broadcast) axis into an AP at position pos."""
    lst = [list(x) for x in ap.ap]
    lst.insert(pos, [0, count])
    return bass.AP(ap.tensor, ap.offset, lst)


F32 = mybir.dt.float32
BF16 = mybir.dt.bfloat16
I16 = mybir.dt.int16

N = 50000
E = 800000
H = 2
CH = 64
IN = 128
OUT = 64
EPS = 1e-5
SLOPE = 0.2
NEG = -30000.0

N_CORES = 8
PER_CORE = 6272            # 49 * 128
NPAD = N_CORES * PER_CORE  # 50176
NBLK = PER_CORE // 128     # 49
REAL_PER_CORE = N // N_CORES  # 6250
NPAD_PER_CORE = PER_CORE - REAL_PER_CORE  # 22
D = 128                    # h channels
ROWE = 256                 # table row elems (bf16) = 512B; [h|als|ald|0..]
LO_END = 32768             # lo window [0, LO_END)
HI_START = NPAD - 32768    # hi window [HI_START, NPAD)
NQ = 4
S_CAP = 64                 # max slots (lo+hi) per group
NB_CAP = 8                 # max blocks per group


# ----------------------------------------------------------------- host prep

def _wrap_idxs(flat):
    """flat [n] int -> dma_gather idx layout [128, n/16] int16 (wrapped in 16
    partitions, replicated across the 8 q7 core groups)."""
    n = flat.shape[0]
    w = flat.reshape(n // 16, 16).T.astype(np.int16)
    return np.tile(w, (8, 1))


def preprocess(edge_index):
    """Build node permutation, per-core rotated ELL grids and masks."""
    src = edge_index[0].astype(np.int64)
    dst = edge_index[1].astype(np.int64)

    indeg = np.bincount(dst, minlength=N)  # real edges only (self via DMA)
    # deal nodes to cores, balancing edges: sort by in-degree, snake-deal
    order = np.argsort(-indeg, kind="stable")
    r = np.arange(N) % (2 * N_CORES)
    core_r = np.where(r < N_CORES, r, 2 * N_CORES - 1 - r)
    core_of = np.empty(N, np.int32)
    core_of[order] = core_r

    def ranks_for(key2):
        new_id = np.empty(N, np.int64)
        for c in range(N_CORES):
            nodes = np.where(core_of == c)[0]
            if key2 is None:
                k = np.lexsort((nodes, -indeg[nodes]))
            else:
                k = np.lexsort((nodes, key2[nodes], -indeg[nodes]))
            new_id[nodes[k]] = c * PER_CORE + np.arange(len(nodes))
        return new_id

    new_id = ranks_for(None)
    # per-dst window-balance key w.r.t. the dst's own core rotation;
    # iterate since the key depends on the ordering it produces
    d_core = core_of[dst]
    for _ in range(3):
        s_new = new_id[src]
        rot_s = (s_new - d_core * PER_CORE) % NPAD
        must_lo_cnt = np.bincount(
            dst, weights=(rot_s < HI_START).astype(np.float64), minlength=N)
        must_hi_cnt = np.bincount(
            dst, weights=(rot_s >= LO_END).astype(np.float64), minlength=N)
        new_id = ranks_for(must_lo_cnt - must_hi_cnt)

    def block_stats(new_id):
        """Per (core, block) rows + window budgets under new_id."""
        ns = new_id[src]
        nd = new_id[dst]
        o = np.argsort(nd, kind="stable")
        ns, nd = ns[o], nd[o]
        starts = np.searchsorted(nd, np.arange(NPAD))
        ends = np.searchsorted(nd, np.arange(NPAD) + 1)
        rows_all = {}
        kcb = np.zeros((N_CORES, NBLK, 2), np.int64)
        for c in range(N_CORES):
            base_c = c * PER_CORE
            for b in range(NBLK):
                ml = 0
                mh = 0
                dmax = 0
                rows = []
                for p in range(128):
                    d_node = base_c + b * 128 + p
                    sl = (ns[starts[d_node]:ends[d_node]] - base_c) % NPAD
                    lo_m = sl[sl < HI_START]
                    hi_m = sl[sl >= LO_END]
                    mid = sl[(sl >= HI_START) & (sl < LO_END)]
                    rows.append((lo_m, hi_m, mid))
                    ml = max(ml, len(lo_m))
                    mh = max(mh, len(hi_m))
                    dmax = max(dmax, len(sl))
                klo = ml
                khi = max(mh, dmax - klo)
                kcb[c, b] = (klo, khi)
                rows_all[(c, b)] = rows
        return rows_all, kcb

    rows_all, kcb = block_stats(new_id)
    KLO = kcb[:, :, 0].max(axis=0)
    KHI = kcb[:, :, 1].max(axis=0)

    tot_slots = int((KLO + KHI).sum() * 128)
    tot_edges = E // N_CORES
    print(f"[prep] slots/core {tot_slots} vs edges/core ~{tot_edges} "
          f"(pad {tot_slots / tot_edges - 1:.1%})  K={int((KLO + KHI).sum())}")

    # group blocks under a slot budget
    groups = []
    cur = []
    cur_s = 0
    for b in range(NBLK):
        sb = int(KLO[b] + KHI[b])
        if cur and (cur_s + sb > S_CAP or len(cur) >= NB_CAP):
            groups.append(cur)
            cur = []
            cur_s = 0
        cur.append(b)
        cur_s += sb
    if cur:
        groups.append(cur)
    print(f"[prep] {len(groups)} groups, sizes {[len(g) for g in groups]}")

    olo = np.concatenate([[0], np.cumsum(KLO)]).astype(int)
    ohi = np.concatenate([[0], np.cumsum(KHI)]).astype(int)
    SLO, SHI = int(KLO.sum()), int(KHI.sum())

    grids_lo = np.zeros((N_CORES, 128, SLO), np.int64)
    grids_hi = np.zeros((N_CORES, 128, SHI), np.int64)
    mlo = np.full((N_CORES, 128, SLO), NEG, np.float32)
    mhi = np.full((N_CORES, 128, SHI), NEG, np.float32)
    for c in range(N_CORES):
        for b in range(NBLK):
            klo, khi = int(KLO[b]), int(KHI[b])
            rows = rows_all[(c, b)]
            for p in range(128):
                lo_m, hi_m, mid = rows[p]
                lo = list(lo_m)
                hi = list(hi_m)
                room = klo - len(lo)
                lo += list(mid[:room])
                hi += list(mid[room:])
                assert len(lo) <= klo and len(hi) <= khi
                g = grids_lo[c, p]
                g[olo[b]:olo[b] + len(lo)] = lo
                g[olo[b] + len(lo):olo[b + 1]] = lo[0] if lo else 0
                g2 = grids_hi[c, p]
                g2[ohi[b]:ohi[b] + len(hi)] = hi
                g2[ohi[b] + len(hi):ohi[b + 1]] = hi[0] if hi else HI_START
                mlo[c, p, olo[b]:olo[b] + len(lo)] = 0.0
                mhi[c, p, ohi[b]:ohi[b] + len(hi)] = 0.0

    # wrap indices for dma_gather: per group, blocks concatenated, slot-major
    # masks combined per group: [mlo_grp | mhi_grp]
    glo_w = np.zeros((N_CORES, 128, 8 * SLO), np.int16)
    ghi_w = np.zeros((N_CORES, 128, 8 * SHI), np.int16)
    mcomb = np.zeros((N_CORES, 128, SLO + SHI), np.float32)
    for c in range(N_CORES):
        for grp in groups:
            fl = []
            fh = []
            for b in grp:
                fl.append(grids_lo[c, :, olo[b]:olo[b + 1]].T.reshape(-1))
                fh.append((grids_hi[c, :, ohi[b]:ohi[b + 1]].T.reshape(-1)
                           - HI_START))
            fl = np.concatenate(fl) if fl else np.zeros(0, np.int64)
            fh = np.concatenate(fh) if fh else np.zeros(0, np.int64)
            b0, b1 = grp[0], grp[-1]
            if len(fl):
                glo_w[c, :, 8 * olo[b0]:8 * olo[b1 + 1]] = _wrap_idxs(fl)
            if len(fh):
                ghi_w[c, :, 8 * ohi[b0]:8 * ohi[b1 + 1]] = _wrap_idxs(fh)
            mo = olo[b0] + ohi[b0]
            slg = olo[b1 + 1] - olo[b0]
            shg = ohi[b1 + 1] - ohi[b0]
            mcomb[c, :, mo:mo + slg] = mlo[c, :, olo[b0]:olo[b1 + 1]]
            mcomb[c, :, mo + slg:mo + slg + shg] = mhi[c, :, ohi[b0]:ohi[b1 + 1]]

    return dict(new_id=new_id, KLO=KLO.tolist(), KHI=KHI.tolist(),
                groups=groups, glo=glo_w, ghi=ghi_w, mcomb=mcomb)


# ----------------------------------------------------------------- builder

def build(KLO, KHI, groups):
    nc = bacc.Bacc(None, target_bir_lowering=False, debug=False,
                   num_devices=N_CORES, num_swdge_queues=NQ)
    SLO, SHI = sum(KLO), sum(KHI)
    olo = np.concatenate([[0], np.cumsum(KLO)]).astype(int).tolist()
    ohi = np.concatenate([[0], np.cumsum(KHI)]).astype(int).tolist()

    xt = nc.dram_tensor("xt", [128, NPAD], BF16, kind="ExternalInput")
    part = nc.dram_tensor("part", [128, 16], F32, kind="ExternalInput")
    gvec = nc.dram_tensor("gvec", [128, 1], F32, kind="ExternalInput")
    bevec = nc.dram_tensor("bevec", [128, 1], F32, kind="ExternalInput")
    srel = nc.dram_tensor("srel", [128, 1], F32, kind="ExternalInput")
    wtmat = nc.dram_tensor("wtmat", [128, 128], F32, kind="ExternalInput")
    emat = nc.dram_tensor("emat", [128, ROWE], F32, kind="ExternalInput")
    mmat = nc.dram_tensor("mmat", [128, 128], F32, kind="ExternalInput")
    biasv = nc.dram_tensor("biasv", [128, 1], F32, kind="ExternalInput")
    glod = nc.dram_tensor("glo", [128, 8 * SLO], I16, kind="ExternalInput")
    ghid = nc.dram_tensor("ghi", [128, 8 * SHI], I16, kind="ExternalInput")
    mcombd = nc.dram_tensor("mcomb", [128, SLO + SHI], F32,
                            kind="ExternalInput")

    outb = nc.dram_tensor("outb", [128, PER_CORE], F32, kind="ExternalOutput")
    parts = nc.dram_tensor("parts", [128, 2], F32, kind="ExternalOutput")

    tbl = nc.dram_tensor("tbl", [NPAD, ROWE], BF16)  # internal

    with tile.TileContext(nc) as tc:
        with (
            tc.tile_pool(name="const", bufs=1) as cpool,
            tc.tile_pool(name="norm", bufs=2) as npool,
            tc.tile_pool(name="tb", bufs=2) as tbpool,
            tc.tile_pool(name="grid", bufs=4) as grpool,
            tc.tile_pool(name="glo", bufs=3) as glopool,
            tc.tile_pool(name="ghi", bufs=3) as ghipool,
            tc.tile_pool(name="self", bufs=6) as stpool,
            tc.tile_pool(name="work", bufs=2) as wpool,
            tc.tile_pool(name="small", bufs=3) as spool,
            tc.tile_pool(name="acc", bufs=1) as apool,
            tc.tile_pool(name="ps", bufs=2, space="PSUM") as pspool,
            tc.tile_pool(name="psw", bufs=1, space="PSUM") as pswpool,
            tc.tile_pool(name="ps2", bufs=2, space="PSUM") as ps2pool,
        ):
            ident = cpool.tile([128, 128], F32, tag="ident")
            make_identity(nc, ident[:])

            # --- BN params ------------------------------------------------
            pt = cpool.tile([128, 16], F32, tag="pt")
            nc.sync.dma_start(pt[:], part.ap())
            gv = cpool.tile([128, 1], F32, tag="gv")
            nc.sync.dma_start(gv[:], gvec.ap())
            bev = cpool.tile([128, 1], F32, tag="bev")
            nc.sync.dma_start(bev[:], bevec.ap())
            sv = cpool.tile([128, 1], F32, tag="sv")
            nc.sync.dma_start(sv[:], srel.ap())

            s1 = cpool.tile([128, 1], F32, tag="s1")
            s2 = cpool.tile([128, 1], F32, tag="s2")
            nc.vector.reduce_sum(s1[:], pt[:, 0:8], axis=mybir.AxisListType.X)
            nc.vector.reduce_sum(s2[:], pt[:, 8:16], axis=mybir.AxisListType.X)
            mu = cpool.tile([128, 1], F32, tag="mu")
            nc.vector.tensor_scalar_mul(mu[:], s1[:], 1.0 / N)
            msq = cpool.tile([128, 1], F32, tag="msq")
            nc.vector.tensor_scalar_mul(msq[:], s2[:], 1.0 / N)
            var = cpool.tile([128, 1], F32, tag="var")
            nc.vector.tensor_tensor(out=var[:], in0=mu[:], in1=mu[:],
                                    op=mybir.AluOpType.mult)
            nc.vector.tensor_tensor(out=var[:], in0=msq[:], in1=var[:],
                                    op=mybir.AluOpType.subtract)
            sd = cpool.tile([128, 1], F32, tag="sd")
            epsT = cpool.tile([128, 1], F32, tag="epsT")
            nc.vector.memset(epsT[:], EPS)
            nc.scalar.activation(sd[:], var[:], mybir.ActivationFunctionType.Sqrt,
                                 bias=epsT[:], scale=1.0)
            ra = cpool.tile([128, 1], F32, tag="ra")
            nc.vector.reciprocal(ra[:], sd[:])
            av = cpool.tile([128, 1], F32, tag="av")
            nc.vector.tensor_tensor(out=av[:], in0=ra[:], in1=gv[:],
                                    op=mybir.AluOpType.mult)
            bv = cpool.tile([128, 1], F32, tag="bv")
            nc.vector.tensor_tensor(out=bv[:], in0=mu[:], in1=av[:],
                                    op=mybir.AluOpType.mult)
            nc.vector.tensor_tensor(out=bv[:], in0=bev[:], in1=bv[:],
                                    op=mybir.AluOpType.subtract)

            wtt = cpool.tile([128, 128], F32, tag="wtt")
            nc.sync.dma_start(wtt[:], wtmat.ap())
            emt = cpool.tile([128, ROWE], F32, tag="emt")
            nc.sync.dma_start(emt[:], emat.ap())
            wep = pswpool.tile([128, ROWE], F32, tag="wep", space="PSUM")
            nc.tensor.matmul(wep[:], lhsT=wtt[:], rhs=emt[:], start=True, stop=True)
            web = cpool.tile([128, ROWE], BF16, tag="web")
            nc.scalar.copy(web[:], wep[:])
            mm = cpool.tile([128, 128], F32, tag="mm")
            nc.sync.dma_start(mm[:], mmat.ap())
            bi = cpool.tile([128, 1], F32, tag="bi")
            nc.sync.dma_start(bi[:], biasv.ap())
            slp = cpool.tile([128, 1], F32, tag="slp")
            nc.vector.memset(slp[:], SLOPE)

            # --- table build: T[r] = prelu(av*x+bv)^T @ [W|a] -------------
            # row layout (bf16): [h(128) | als(2) | ald(2) | pad]
            CH_N = 1024
            for r0 in range(0, NPAD, CH_N):
                xn = npool.tile([128, CH_N], BF16, tag="xn")
                nc.sync.dma_start(xn[:], xt.ap()[:, r0:r0 + CH_N])
                u = npool.tile([128, CH_N], BF16, tag="u")
                nc.scalar.activation(u[:], xn[:],
                                     mybir.ActivationFunctionType.Prelu,
                                     bias=bv[:], scale=av[:], alpha=sv[:])
                hbt = tbpool.tile([128, 8 * ROWE], BF16, tag="hbt")
                hbt3 = hbt[:].rearrange("p (i e) -> p i e", e=ROWE)
                for q in range(4):
                    hp = pspool.tile([128, 2 * ROWE], F32, tag="hp",
                                     space="PSUM")
                    hp3 = hp[:].rearrange("p (i e) -> p i e", e=ROWE)
                    for t in range(2):
                        rr = (2 * q + t) * 128
                        nc.tensor.matmul(hp3[:, t, :], lhsT=u[:, rr:rr + 128],
                                         rhs=web[:], start=True, stop=True)
                    nc.scalar.copy(hbt3[:, 2 * q:2 * q + 2, 0:D + 2 * H],
                                   hp3[:, :, 0:D + 2 * H])
                out_ap = bass.AP(tbl, r0 * ROWE,
                                 [[ROWE, 128], [128 * ROWE, 8], [1, ROWE]])
                nc.sync.dma_start(out_ap, hbt3)

            # --- per-group aggregation ------------------------------------
            pacc = apool.tile([128, 2], F32, tag="pacc")
            nc.vector.memset(pacc[:], 0.0)
            qn = 0
            for grp in groups:
                nB = len(grp)
                b0 = grp[0]
                klo_g = [KLO[b] for b in grp]
                khi_g = [KHI[b] for b in grp]
                S_lo = sum(klo_g)
                S_hi = sum(khi_g)
                clo = np.concatenate([[0], np.cumsum(klo_g)]).astype(int).tolist()
                chi = np.concatenate([[0], np.cumsum(khi_g)]).astype(int).tolist()

                # self rows: dst rows of this group's blocks (contiguous)
                st = stpool.tile([128, nB * ROWE], BF16, tag="st")
                st3 = st[:].rearrange("p (i e) -> p i e", e=ROWE)
                in_ap = bass.AP(tbl, (b0 * 128) * ROWE,
                                [[ROWE, 128], [128 * ROWE, nB], [1, ROWE]])
                nc.sync.dma_start(st3, in_ap)
                # self als/ald bf16 views [p, nB, 2]
                st_als = bass.AP(st.tensor, st[:].offset + D,
                                 [list(st[:].ap[0]), [ROWE, nB], [1, 2]])
                st_ald = bass.AP(st.tensor, st[:].offset + D + H,
                                 [list(st[:].ap[0]), [ROWE, nB], [1, 2]])

                uu = wpool.tile([128, nB * D], F32, tag="uu")
                uu3 = uu[:].rearrange("p (i c) -> p i c", c=D)
                uh = wpool.tile([128, nB * D], F32, tag="uh")
                uh3 = uh[:].rearrange("p (i c) -> p i c", c=D)
                den = spool.tile([128, nB * H], F32, tag="den")
                den3 = den[:].rearrange("p (i h) -> p i h", h=H)
                dhi = spool.tile([128, nB * H], F32, tag="dhi")
                dhi3 = dhi[:].rearrange("p (i h) -> p i h", h=H)

                # one combined gathered tile: [lo slots | hi slots]
                S = S_lo + S_hi
                mo = int(olo[b0] + ohi[b0])
                g = glopool.tile([128, S * ROWE], BF16, tag="g")
                g3 = g[:].rearrange("p (k e) -> p k e", e=ROWE)
                mkt = grpool.tile([128, S], F32, tag="mkt")
                nc.sync.dma_start(mkt[:], mcombd.ap()[:, mo:mo + S])
                if S_lo:
                    owin = int(olo[b0])
                    glt = grpool.tile([128, 8 * S_lo], I16, tag="gltlo")
                    nc.sync.dma_start(
                        glt[:], glod.ap()[:, 8 * owin:8 * (owin + S_lo)])
                    nc.gpsimd.dma_gather(
                        out_ap=g3[:, 0:S_lo, :], in_ap=tbl.ap()[0:LO_END, :],
                        idxs_ap=glt[:], num_idxs=128 * S_lo,
                        num_idxs_reg=128 * S_lo, elem_size=ROWE,
                        single_packet=False, queue_num=qn % NQ)
                    qn += 1
                if S_hi:
                    owin = int(ohi[b0])
                    ght = grpool.tile([128, 8 * S_hi], I16, tag="glthi")
                    nc.sync.dma_start(
                        ght[:], ghid.ap()[:, 8 * owin:8 * (owin + S_hi)])
                    nc.gpsimd.dma_gather(
                        out_ap=g3[:, S_lo:S, :],
                        in_ap=tbl.ap()[HI_START:NPAD, :],
                        idxs_ap=ght[:], num_idxs=128 * S_hi,
                        num_idxs_reg=128 * S_hi, elem_size=ROWE,
                        single_packet=False, queue_num=qn % NQ)
                    qn += 1

                # ee = als + mask (+ald per block); prelu; exp -> bf16
                ee = wpool.tile([128, S * H], F32, tag="ee")
                ee3 = ee[:].rearrange("p (k h) -> p k h", h=H)
                g_als = bass.AP(g.tensor, g[:].offset + D,
                                [list(g[:].ap[0]), [ROWE, S], [1, 2]])
                nc.vector.tensor_tensor(out=ee3, in0=g_als,
                                        in1=_bc(mkt[:], 2, H),
                                        op=mybir.AluOpType.add)
                for gi in range(nB):
                    ald_b0 = bass.AP(st.tensor,
                                     st[:].offset + D + H + gi * ROWE,
                                     [list(st[:].ap[0]), [0, 1], [1, 2]])
                    for lo0, kb in ((clo[gi], klo_g[gi]),
                                    (S_lo + chi[gi], khi_g[gi])):
                        if kb == 0:
                            continue
                        ald_b = bass.AP(ald_b0.tensor, ald_b0.offset,
                                        [list(st[:].ap[0]), [0, kb], [1, 2]])
                        nc.vector.tensor_tensor(
                            out=ee3[:, lo0:lo0 + kb, :],
                            in0=ee3[:, lo0:lo0 + kb, :],
                            in1=ald_b, op=mybir.AluOpType.add)
                nc.scalar.activation(ee[:], ee[:],
                                     mybir.ActivationFunctionType.Prelu,
                                     alpha=slp[:])
                ex = wpool.tile([128, S * H], BF16, tag="ex")
                nc.scalar.activation(ex[:], ee[:],
                                     mybir.ActivationFunctionType.Exp)

                # in-place alpha multiply on gathered h
                gh = bass.AP(g.tensor, g[:].offset,
                             [list(g[:].ap[0]), [ROWE, S], [CH, H], [1, CH]])
                exb = bass.AP(ex.tensor, ex[:].offset,
                              [list(ex[:].ap[0]), [H, S], [1, H], [0, CH]])
                nc.vector.tensor_tensor(out=gh, in0=gh, in1=exb,
                                        op=mybir.AluOpType.mult)

                # per block: den + aggregation for both slot ranges
                for gi in range(nB):
                    for lo0, kb, denf, uuo in (
                        (clo[gi], klo_g[gi], den, uu3),
                        (S_lo + chi[gi], khi_g[gi], dhi, uh3),
                    ):
                        if kb == 0:
                            nc.vector.memset(denf[:, gi * H:gi * H + H], 0.0)
                            nc.vector.memset(uuo[:, gi, :], 0.0)
                            continue
                        exv = bass.AP(ex.tensor, ex[:].offset + lo0 * H,
                                      [list(ex[:].ap[0]), [1, H], [H, kb]])
                        nc.vector.tensor_reduce(
                            denf[:, gi * H:gi * H + H], exv,
                            axis=mybir.AxisListType.X,
                            op=mybir.AluOpType.add)
                        ghv = bass.AP(g.tensor, g[:].offset + lo0 * ROWE,
                                      [list(g[:].ap[0]), [1, D], [ROWE, kb]])
                        nc.vector.tensor_reduce(
                            uuo[:, gi, :], ghv, axis=mybir.AxisListType.X,
                            op=mybir.AluOpType.add)

                # self contribution: e = als+ald, prelu, exp; den & numerator
                es = spool.tile([128, nB * H], F32, tag="es")
                es3 = es[:].rearrange("p (i h) -> p i h", h=H)
                nc.vector.tensor_tensor(out=es3, in0=st_als, in1=st_ald,
                                        op=mybir.AluOpType.add)
                nc.scalar.activation(es[:], es[:],
                                     mybir.ActivationFunctionType.Prelu,
                                     alpha=slp[:])
                exs = spool.tile([128, nB * H], BF16, tag="exs")
                exs3 = exs[:].rearrange("p (i h) -> p i h", h=H)
                nc.scalar.activation(exs[:], es[:],
                                     mybir.ActivationFunctionType.Exp)

                # den total + reciprocal
                nc.vector.tensor_tensor(out=den[:], in0=den[:], in1=dhi[:],
                                        op=mybir.AluOpType.add)
                nc.vector.tensor_tensor(out=den3, in0=den3, in1=exs3,
                                        op=mybir.AluOpType.add)
                rden = spool.tile([128, nB * H], F32, tag="rden")
                nc.vector.reciprocal(rden[:], den[:])
                rden3 = rden[:].rearrange("p (i h) -> p i h", h=H)

                # self numerator: st.h *= exs ; uu += uh + st.h
                sth = bass.AP(st.tensor, st[:].offset,
                              [list(st[:].ap[0]), [ROWE, nB], [CH, H],
                               [1, CH]])
                exsb = bass.AP(exs.tensor, exs[:].offset,
                               [list(exs[:].ap[0]), [H, nB], [1, H],
                                [0, CH]])
                nc.vector.tensor_tensor(out=sth, in0=sth, in1=exsb,
                                        op=mybir.AluOpType.mult)
                nc.vector.tensor_tensor(out=uu[:], in0=uu[:], in1=uh[:],
                                        op=mybir.AluOpType.add)
                sthv = bass.AP(st.tensor, st[:].offset,
                               [list(st[:].ap[0]), [ROWE, nB], [1, D]])
                nc.vector.tensor_tensor(out=uu3, in0=uu3, in1=sthv,
                                        op=mybir.AluOpType.add)

                # scale by 1/den
                uu4 = bass.AP(uu.tensor, uu[:].offset,
                              [list(uu[:].ap[0]), [D, nB], [CH, H], [1, CH]])
                rdb = bass.AP(rden.tensor, rden[:].offset,
                              [list(rden[:].ap[0]), [H, nB], [1, H], [0, CH]])
                nc.vector.tensor_tensor(out=uu4, in0=uu4, in1=rdb,
                                        op=mybir.AluOpType.mult)

                # tail per block: transpose, +bias, head-mix, collect
                obt = wpool.tile([128, nB * D], F32, tag="obt")
                obt3 = obt[:].rearrange("p (i c) -> p i c", c=D)
                for gi in range(nB):
                    utp = ps2pool.tile([128, 128], F32, tag="utp",
                                       space="PSUM")
                    nc.tensor.transpose(utp[:], uu3[:, gi, :], ident[:])
                    uts = spool.tile([128, 128], F32, tag="uts")
                    nc.vector.tensor_scalar(out=uts[:], in0=utp[:],
                                            scalar1=bi[:], scalar2=None,
                                            op0=mybir.AluOpType.add)
                    otp = ps2pool.tile([128, 128], F32, tag="otp",
                                       space="PSUM")
                    nc.tensor.matmul(otp[:], lhsT=mm[:], rhs=uts[:],
                                     start=True, stop=True)
                    nc.scalar.copy(obt3[:, gi, :], otp[:])

                # partials
                sq = wpool.tile([128, nB * D], F32, tag="sq")
                nc.scalar.square(sq[:], obt[:])
                rs = spool.tile([128, 2], F32, tag="rs")
                nc.vector.reduce_sum(rs[:, 0:1], obt[:],
                                     axis=mybir.AxisListType.X)
                nc.vector.reduce_sum(rs[:, 1:2], sq[:],
                                     axis=mybir.AxisListType.X)
                nc.vector.tensor_tensor(out=pacc[:], in0=pacc[:], in1=rs[:],
                                        op=mybir.AluOpType.add)

                nc.sync.dma_start(
                    outb.ap()[:, b0 * 128:b0 * 128 + nB * D], obt[:])

            nc.sync.dma_start(parts.ap(), pacc[:])

    # align each gather's SWDGE queue with its Tile-assigned DMASW sem lane
    for bb in nc.main_func.blocks:
        for ins in bb.instructions:
            if isinstance(ins, mybir.InstDMAGatherAnt):
                nm = _IDX_TO_PROC.get(ins.bass_scheduled_proc, "")
                if nm.startswith("DMASW"):
                    ins.queue_num = int(nm[5:]) % NQ

    nc.compile()
    return nc


# ----------------------------------------------------------------- driver

_TRACE = bool(os.environ.get("KERNEL_TRACE"))
LAST_EXEC_NS = []


def kernel(x, edge_index, W0, a_src0, a_dst0, b0, g0, be0,
           W1, a_src1, a_dst1, b1, g1, be1,
           W2, a_src2, a_dst2, b2):
    global LAST_EXEC_NS
    LAST_EXEC_NS = []
    prep = preprocess(np.asarray(edge_index))
    new_id = prep["new_id"]

    nc = build(prep["KLO"], prep["KHI"], prep["groups"])

    xp = np.zeros((NPAD, IN), np.float32)
    xp[new_id] = np.asarray(x, np.float32)

    eye = np.eye(128, dtype=np.float32)
    mix2 = np.zeros((128, 128), np.float32)
    mix2[0:64, 0:64] = 0.5 * np.eye(64)
    mix2[64:128, 0:64] = 0.5 * np.eye(64)

    layers = [
        dict(W=W0, a_src=a_src0, a_dst=a_dst0,
             beta=np.asarray(b0, np.float32),
             g=np.full(128, np.sqrt(EPS), np.float32),
             be=np.zeros(128, np.float32), s=1.0, mix=eye),
        dict(W=W1, a_src=a_src1, a_dst=a_dst1,
             beta=np.asarray(b1, np.float32),
             g=np.asarray(g0, np.float32), be=np.asarray(be0, np.float32),
             s=0.0, mix=eye),
        dict(W=W2, a_src=a_src2, a_dst=a_dst2,
             beta=np.concatenate([np.asarray(b2, np.float32),
                                  np.asarray(b2, np.float32)]),
             g=np.asarray(g1, np.float32), be=np.asarray(be1, np.float32),
             s=0.0, mix=mix2),
    ]

    # pad column ids (global padded coords), per core
    pad_cols = np.concatenate(
        [np.arange(c * PER_CORE + REAL_PER_CORE, (c + 1) * PER_CORE)
         for c in range(N_CORES)])

    xt_cur = np.ascontiguousarray(xp.T).astype(np.float32)  # [128, NPAD]
    part_cur = np.zeros((128, 16), np.float32)

    outf = None
    for li, L in enumerate(layers):
        emat = np.zeros((128, ROWE), np.float32)
        emat[:, 0:128] = np.eye(128, dtype=np.float32)
        a_s = np.asarray(L["a_src"], np.float32)
        a_d = np.asarray(L["a_dst"], np.float32)
        for hh in range(H):
            emat[hh * CH:(hh + 1) * CH, D + hh] = a_s[hh]
            emat[hh * CH:(hh + 1) * CH, D + H + hh] = a_d[hh]

        # host-side BN params (match device math) for pad column values
        if li == 0:
            av = np.ones(128, np.float32)
            bv = np.zeros(128, np.float32)
        else:
            mu = part_cur[:, 0:8].sum(axis=1) / N
            msq = part_cur[:, 8:16].sum(axis=1) / N
            var = msq - mu * mu
            av = L["g"] / np.sqrt(var + EPS)
            bv = L["be"] - mu * av
        xt_cur[:, pad_cols] = (-bv / av)[:, None]

        import ml_dtypes
        in_maps = []
        for c in range(N_CORES):
            xt_rot = np.roll(xt_cur, -c * PER_CORE, axis=1)
            in_maps.append(dict(
                xt=np.ascontiguousarray(xt_rot).astype(ml_dtypes.bfloat16),
                part=part_cur,
                gvec=np.asarray(L["g"], np.float32).reshape(128, 1),
                bevec=np.asarray(L["be"], np.float32).reshape(128, 1),
                srel=np.full((128, 1), L["s"], np.float32),
                wtmat=np.ascontiguousarray(np.asarray(L["W"], np.float32).T),
                emat=emat,
                mmat=np.asarray(L["mix"], np.float32),
                biasv=L["beta"].reshape(128, 1),
                glo=prep["glo"][c],
                ghi=prep["ghi"][c],
                mcomb=prep["mcomb"][c],
            ))

        res = bass_utils.run_bass_kernel_spmd(
            nc, in_maps, core_ids=list(range(N_CORES)), trace=_TRACE)
        if _TRACE and res.exec_time_ns:
            LAST_EXEC_NS.append(res.exec_time_ns)

        xt_cur = np.concatenate(
            [np.asarray(res.results[c]["outb"], np.float32)
             for c in range(N_CORES)], axis=1)
        # partials: subtract the pad columns' exact beta contribution
        beta = L["beta"]
        part_pairs = [np.asarray(res.results[c]["parts"], np.float32)
                      for c in range(N_CORES)]
        sums = np.stack([p[:, 0] - NPAD_PER_CORE * beta
                         for p in part_pairs], axis=1)
        sqs = np.stack([p[:, 1] - NPAD_PER_CORE * beta * beta
                        for p in part_pairs], axis=1)
        part_cur = np.concatenate([sums, sqs], axis=1).astype(np.float32)
        if li == 2:
            outf = xt_cur

    out = np.zeros((N, OUT), np.float32)
    out[np.arange(N)] = outf[:OUT, :].T[new_id]
    return out
